# revision 1
# baseline (speedup 1.0000x reference)
"""Trainium2 Bass kernel for the sparse-attention ('interact' mask) transformer block.

Reference computation (B=4, N=1569, C=768, H=12, d=64, Dff=3072, F=9):
    h   = LN(x);  qkv = h @ qkv_w.T;  sparse attention (spatial rows attend
    only to the 9 temporal tokens, temporal rows attend to the 1560 spatial
    tokens, CLS also to itself);  out = attn @ proj_w.T + proj_b;
    return out + MLP(LN(out))

Sharding: 8 cores = 4 batches x 2 halves. Each core owns one batch's half of
the 1560 spatial tokens (780) plus a replicated copy of the 9 temporal
tokens; local token layout is [780 spatial | 9 temporal].  The only
communication is a pairwise AllReduce(add) of flash-style partial softmax
stats (l2 [108,1], O2 [9,768]) for the 9 temporal query rows.

On-chip layout is feature-major [C, tokens]; the host pre-transposes x and
all weights (pure data movement, part of sharding).  Matmuls run as fp32r
(full PE rate); LN statistics come from ones-matmuls; softmax skips the max
subtraction (scores are O(1) here, exp is safe in fp32).
"""

import numpy as np
import sys
from contextlib import ExitStack

sys.path.insert(0, '/opt/trn_rl_repo')

import concourse.bass as bass
import concourse.bacc as bacc
import concourse.tile as tile
from concourse import mybir
from concourse.bass_utils import run_bass_kernel_spmd

# ---------------- problem constants (hardcoded per contract) ----------------
B, N, C = 4, 1569, 768
H, D = 12, 64
F = 9                    # temporal tokens (CLS + 8 frames)
DFF = 4 * C              # 3072
NSP = N - F              # 1560 spatial tokens
SPH = NSP // 2           # 780 spatial tokens per core
T = SPH + F + 1          # 790 local cols: [780 spatial | 9 temporal | 1 zero pad]
                         # (pad keeps every fp32r matmul moving-dim even)
NCH = C // 128           # 6 feature chunks
NCH_FF = DFF // 128      # 24 hidden chunks
NTB = (T + 127) // 128   # 7 token blocks (last = 21 rows)
SCALE = D ** -0.5

FP32 = mybir.dt.float32
FP32R = mybir.dt.float32r
BF16 = mybir.dt.bfloat16

# free-dim tiles for matmul moving operand (<=512 fp32 / PSUM bank)
T_TILES = [(0, 512), (512, SPH), (SPH, T)]          # [0:512) [512:780) [780:790)
T_TILES_SP = [(0, 512), (512, SPH)]                 # spatial-only part


def _r(ap):
    """View an fp32 AP as fp32r for full-rate PE matmuls."""
    return ap.bitcast(FP32R)


def build_kernel():
    nc = bacc.Bacc("TRN2", target_bir_lowering=False, debug=False,
                   num_devices=8)

    # ---------------- DRAM I/O ----------------
    xT = nc.dram_tensor("xT", [C, T], FP32R, kind="ExternalInput")
    qkvWt = nc.dram_tensor("qkvWt", [C, 3 * C], BF16, kind="ExternalInput")
    projWt = nc.dram_tensor("projWt", [C, C], BF16, kind="ExternalInput")
    fc1Wt = nc.dram_tensor("fc1Wt", [C, DFF], BF16, kind="ExternalInput")
    fc2Wt = nc.dram_tensor("fc2Wt", [DFF, C], BF16, kind="ExternalInput")
    # [C,2]: col0 = ln2_g, col1 = ln2_b ; biases as [dim,1]
    gb = nc.dram_tensor("gb", [C, 2], FP32, kind="ExternalInput")
    projB = nc.dram_tensor("projB", [C, 1], FP32, kind="ExternalInput")
    fc1B = nc.dram_tensor("fc1B", [DFF, 1], FP32, kind="ExternalInput")
    fc2B = nc.dram_tensor("fc2B", [C, 1], FP32, kind="ExternalInput")
    ones = nc.dram_tensor("ones", [128, 1], FP32R, kind="ExternalInput")
    onesrow = nc.dram_tensor("onesrow", [1, 128], FP32R, kind="ExternalInput")
    headsel = nc.dram_tensor("headsel", [H, C], FP32R, kind="ExternalInput")
    bd9 = nc.dram_tensor("bd9", [H * F, H], FP32R, kind="ExternalInput")
    ident = nc.dram_tensor("ident", [128, 128], FP32, kind="ExternalInput")
    # e00mask [1,108]: is_even at positions h*9, else 0 (0 everywhere on odd cores)
    e00mask = nc.dram_tensor("e00mask", [1, H * F], FP32, kind="ExternalInput")
    zeros = nc.dram_tensor("zeros", [128, C], FP32R, kind="ExternalInput")
    outT = nc.dram_tensor("outT", [C, T], FP32, kind="ExternalOutput")

    with tile.TileContext(nc) as tc, ExitStack() as ctx:
        act = ctx.enter_context(tc.tile_pool(name="act", bufs=1))
        big = ctx.enter_context(tc.tile_pool(name="big", bufs=1))
        wpool = ctx.enter_context(tc.tile_pool(name="w", bufs=6))
        wpool2 = ctx.enter_context(tc.tile_pool(name="w2", bufs=25))
        small = ctx.enter_context(tc.tile_pool(name="small", bufs=1))
        stage = ctx.enter_context(tc.tile_pool(name="stage", bufs=2))
        psmm = ctx.enter_context(tc.tile_pool(name="psmm", bufs=3, space="PSUM"))
        psst = ctx.enter_context(tc.tile_pool(name="psst", bufs=3, space="PSUM"))
        pso2 = ctx.enter_context(tc.tile_pool(name="pso2", bufs=2, space="PSUM"))
        dram = ctx.enter_context(tc.tile_pool(name="dram", bufs=1, space="DRAM"))

        # ---------------- constants / biases ----------------
        ones_t = small.tile([128, 1], FP32R, tag="ones", name="ones")
        nc.sync.dma_start(ones_t[:], ones[:])
        onesrow_t = small.tile([1, 128], FP32R, tag="onesrow", name="onesrow")
        nc.sync.dma_start(onesrow_t[:], onesrow[:])
        headsel_t = small.tile([H, C], FP32R, tag="headsel", name="headsel")
        nc.sync.dma_start(headsel_t[:], headsel[:])
        bd9_t = small.tile([H * F, H], FP32R, tag="bd9", name="bd9")
        nc.sync.dma_start(bd9_t[:], bd9[:])
        id_t = small.tile([128, 128], FP32, tag="ident", name="ident")
        nc.sync.dma_start(id_t[:], ident[:])
        e00_t = small.tile([1, H * F], FP32, tag="e00", name="e00")
        nc.sync.dma_start(e00_t[:], e00mask[:])
        gb_t = [small.tile([128, 2], FP32, tag=f"gb{ci}", name=f"gb{ci}") for ci in range(NCH)]
        for ci in range(NCH):
            nc.sync.dma_start(gb_t[ci][:], gb[ci * 128:(ci + 1) * 128, :])
        pb_t = [small.tile([128, 1], FP32, tag=f"pb{ci}", name=f"pb{ci}") for ci in range(NCH)]
        for ci in range(NCH):
            nc.sync.dma_start(pb_t[ci][:], projB[ci * 128:(ci + 1) * 128, :])
        f1b_t = [small.tile([128, 1], FP32, tag=f"f1b{ci}", name=f"f1b{ci}") for ci in range(NCH_FF)]
        for ci in range(NCH_FF):
            nc.sync.dma_start(f1b_t[ci][:], fc1B[ci * 128:(ci + 1) * 128, :])
        f2b_t = [small.tile([128, 1], FP32, tag=f"f2b{ci}", name=f"f2b{ci}") for ci in range(NCH)]
        for ci in range(NCH):
            nc.sync.dma_start(f2b_t[ci][:], fc2B[ci * 128:(ci + 1) * 128, :])

        # ---------------- load x ----------------
        x_t = [act.tile([128, T], FP32R, tag=f"x{ci}", name=f"x{ci}") for ci in range(NCH)]
        for ci in range(NCH):
            nc.sync.dma_start(x_t[ci][:], xT[ci * 128:(ci + 1) * 128, :])

        # =========================================================
        # helper: layernorm stats + apply  (feature-major)
        #   in:  src chunks [128, T] x6     out: dst chunks [128, T] x6
        # =========================================================
        def layer_norm_fm(src, dst, scratch_tag, bc_a, bc_b):
            """LN over features (partition dim).  src/dst: 6 chunks [128,T].
            bc_a/bc_b: [128,T] scratch tiles for broadcast alpha/beta."""
            # x^2 into scratch
            sq = [act.tile([128, T], FP32R, tag=f"{scratch_tag}{ci}", name=f"{scratch_tag}{ci}")
                  for ci in range(NCH)]
            for ci in range(NCH):
                nc.scalar.activation(sq[ci][:], src[ci][:],
                                     mybir.ActivationFunctionType.Square)
            # LN scalar math stays in PSUM at partition 0 (ACT/DVE cannot
            # shift partitions; separate SBUF rows would break that rule).
            al_t = small.tile([1, T], FP32R, tag=f"{scratch_tag}_al", name=f"{scratch_tag}_al")
            be_t = small.tile([1, T], FP32R, tag=f"{scratch_tag}_be", name=f"{scratch_tag}_be")
            for (t0, t1) in T_TILES:
                w = t1 - t0
                ps = psst.tile([1, 512], FP32, tag="stat", name="stat")
                for ci in range(NCH):
                    nc.tensor.matmul(ps[:, :w], ones_t[:],
                                     src[ci][:, t0:t1],
                                     start=(ci == 0), stop=(ci == NCH - 1))
                ps2 = psst.tile([1, 512], FP32, tag="stat", name="stat2")
                for ci in range(NCH):
                    nc.tensor.matmul(ps2[:, :w], ones_t[:],
                                     sq[ci][:, t0:t1],
                                     start=(ci == 0), stop=(ci == NCH - 1))
                # scalar chain: one PSUM operand per op; intermediates in SBUF
                rowA = small.tile([1, 512], FP32, tag="lnA", name="lnA")
                rowB = small.tile([1, 512], FP32, tag="lnB", name="lnB")
                nc.vector.tensor_scalar_mul(ps[:, :w], ps[:, :w], 1.0 / C)
                nc.scalar.copy(rowA[:, :w], ps[:, :w])                  # mean
                nc.vector.tensor_scalar_mul(ps2[:, :w], ps2[:, :w], 1.0 / C)
                nc.vector.tensor_mul(rowB[:, :w], rowA[:, :w], rowA[:, :w])
                nc.vector.tensor_sub(ps2[:, :w], ps2[:, :w], rowB[:, :w])
                nc.vector.tensor_scalar_add(ps2[:, :w], ps2[:, :w], 1e-5)
                nc.scalar.activation(rowB[:, :w], ps2[:, :w],
                                     mybir.ActivationFunctionType.Sqrt)
                with nc.allow_low_precision(reason="fp32r LN alpha rounding intended"):
                    nc.vector.reciprocal(al_t[:, t0:t1], rowB[:, :w])
                nc.vector.tensor_mul(rowB[:, :w], rowA[:, :w], al_t[:, t0:t1])
                with nc.allow_low_precision(reason="fp32r LN beta rounding intended"):
                    nc.vector.tensor_scalar_mul(be_t[:, t0:t1], rowB[:, :w], -1.0)
            # broadcast alpha/beta across partitions via K=1 ones-matmul
            for (srow, bct) in ((al_t, bc_a), (be_t, bc_b)):
                for (t0, t1) in T_TILES:
                    psb = psmm.tile([128, 512], FP32, tag="mm", name="mm")
                    nc.tensor.matmul(psb[:, :t1 - t0], onesrow_t[:],
                                     srow[:, t0:t1],
                                     start=True, stop=True)
                    nc.scalar.copy(bct[:, t0:t1], psb[:, :t1 - t0])
            # apply: dst = (src*alpha + beta) * g + b
            for ci in range(NCH):
                nc.vector.tensor_mul(dst[ci][:], src[ci][:], bc_a[:])
                nc.vector.tensor_tensor(dst[ci][:], dst[ci][:], bc_b[:],
                                        op=mybir.AluOpType.add)
                nc.vector.tensor_scalar(dst[ci][:], dst[ci][:],
                                        gb_t[ci][:, 0:1], gb_t[ci][:, 1:2],
                                        op0=mybir.AluOpType.mult,
                                        op1=mybir.AluOpType.add)

        # =========================================================
        # STAGE A: LN1 + qkv
        # =========================================================
        h_t = [act.tile([128, T], BF16, tag=f"h{ci}", name=f"h{ci}") for ci in range(NCH)]
        bc_a = small.tile([128, T], FP32, tag="bca", name="bca")
        bc_b = small.tile([128, T], FP32, tag="bcb", name="bcb")
        layer_norm_fm(x_t, h_t, "k", bc_a, bc_b)  # scratch shares k-tag slots

        # q, k feature-major [C, T]; v token-major [T, C]
        q_t = [act.tile([128, T], FP32R, tag=f"q{ci}", name=f"q{ci}") for ci in range(NCH)]
        k_t = [act.tile([128, T], FP32R, tag=f"k{ci}", name=f"k{ci}") for ci in range(NCH)]
        v_t = [big.tile([128, C], FP32R, tag=f"v{tb}", name=f"v{tb}") for tb in range(NTB)]

        # q,k: for each 512-wide cout group load W [128,512] x6, then mm per 128-col block
        for qk in range(2):          # 0 = q, 1 = k
            dstl = q_t if qk == 0 else k_t
            for cg in range(0, C, 512):
                gw = min(512, C - cg)
                wts = [wpool.tile([128, 512], BF16, tag="w", name="w") for _ in range(NCH)]
                for ci in range(NCH):
                    nc.sync.dma_start(
                        wts[ci][:, :gw], qkvWt[ci * 128:(ci + 1) * 128,
                                               qk * C + cg: qk * C + cg + gw])
                for co in range(gw // 128):  # 128-col blocks within the group
                    cout = cg + co * 128
                    for (t0, t1) in T_TILES:
                        ps = psmm.tile([128, 512], FP32, tag="mm", name="mm")
                        for ci in range(NCH):
                            nc.tensor.matmul(
                                ps[:, :t1 - t0],
                                wts[ci][:, co * 128:(co + 1) * 128],
                                h_t[ci][:, t0:t1],
                                start=(ci == 0), stop=(ci == NCH - 1))
                        nc.scalar.copy(dstl[cout // 128][:, t0:t1], ps[:, :t1 - t0])

        # v token-major: for tok block: lhsT = h chunk [128cin, tb 128], rhs = W [128,512]
        for cg in range(0, C, 512):
            gw = min(512, C - cg)
            wts = [wpool.tile([128, 512], BF16, tag="w", name="w") for _ in range(NCH)]
            for ci in range(NCH):
                nc.sync.dma_start(
                    wts[ci][:, :gw], qkvWt[ci * 128:(ci + 1) * 128,
                                           2 * C + cg: 2 * C + cg + gw])
            for tb in range(NTB):
                p0, p1 = tb * 128, min((tb + 1) * 128, T)
                ps = psmm.tile([128, 512], FP32, tag="mm", name="mm")
                for ci in range(NCH):
                    nc.tensor.matmul(ps[:p1 - p0, :gw],
                                     h_t[ci][:, p0:p1],
                                     wts[ci][:, :gw],
                                     start=(ci == 0), stop=(ci == NCH - 1))
                nc.scalar.copy(v_t[tb][:p1 - p0, cg:cg + gw], ps[:p1 - p0, :gw])

        # =========================================================
        # STAGE B: sparse attention
        # =========================================================
        # temporal-token copies with base-partition 0 (tokens 780..789 live in
        # token block 6 at partitions 12..21)
        vtmp = small.tile([F, C], FP32R, tag="vtmp", name="vtmp")
        nc.sync.dma_start(vtmp[:], v_t[6][12:12 + F, :])

        # block-diag K_bd, Q_bd [128, 108] x6 : head h occupies rows (h%2)*64..,
        # cols h*9..h*9+9, from the temporal slice of k / q
        kbd = [small.tile([128, H * F], FP32R, tag=f"kbd{ci}", name=f"kbd{ci}") for ci in range(NCH)]
        qbd = [small.tile([128, H * F], FP32R, tag=f"qbd{ci}", name=f"qbd{ci}") for ci in range(NCH)]
        for ci in range(NCH):
            nc.sync.dma_start(kbd[ci][:], zeros[:, :H * F])
            nc.sync.dma_start(qbd[ci][:], zeros[:, :H * F])
        for h in range(H):
            ci, po = h // 2, (h % 2) * 64
            nc.vector.tensor_copy(kbd[ci][po:po + 64, h * F:(h + 1) * F],
                                  k_t[ci][po:po + 64, SPH:SPH + F])
            nc.vector.tensor_copy(qbd[ci][po:po + 64, h * F:(h + 1) * F],
                                  q_t[ci][po:po + 64, SPH:SPH + F])

        # ---- S1/P1: all local queries vs 9 temporal keys -> [108, T]
        p1 = small.tile([H * F, T], FP32R, tag="p1", name="p1")
        for (t0, t1) in T_TILES:
            ps = psmm.tile([128, 512], FP32, tag="mm", name="mm")
            for ci in range(NCH):
                nc.tensor.matmul(ps[:H * F, :t1 - t0], kbd[ci][:],
                                 q_t[ci][:, t0:t1],
                                 start=(ci == 0), stop=(ci == NCH - 1))
            nc.scalar.activation(p1[:, t0:t1], ps[:H * F, :t1 - t0],
                                 mybir.ActivationFunctionType.Exp, scale=SCALE)

        # lsp[h,t] = sum_j P1[h*9+j, t]  (fp32 matmul with block-diag ones)
        lsp = small.tile([H, T], FP32R, tag="lsp", name="lsp")
        for (t0, t1) in T_TILES:
            ps = psst.tile([12, 512], FP32, tag="stat", name="lspps")
            nc.tensor.matmul(ps[:, :t1 - t0], bd9_t[:], p1[:, t0:t1],
                             start=True, stop=True)
            nc.scalar.copy(lsp[:, t0:t1], ps[:, :t1 - t0])
        rlsp = lsp
        with nc.allow_low_precision(reason="fp32r rounding of softmax recip is intentional"):
            nc.vector.reciprocal(rlsp[:], lsp[:])

        # ---- O1: spatial attention out via block-diag v_tmp, normalized on copy
        # vtmp_bd [108, 768]: rows (h,j), head h's v at cols h*64..h*64+64
        vtmp_bd = small.tile([H * F, C], FP32R, tag="vtmpbd", name="vtmpbd")
        nc.sync.dma_start(vtmp_bd[:], zeros[:H * F, :])
        for h in range(H):
            nc.sync.dma_start(vtmp_bd[h * F:(h + 1) * F, h * 64:(h + 1) * 64],
                              vtmp[:, h * 64:(h + 1) * 64])
        attnout = [act.tile([128, T], BF16, tag=f"x{ci}", name=f"attn{ci}") for ci in range(NCH)]  # reuse x slots
        for ci in range(NCH):
            for (t0, t1) in T_TILES_SP:
                ps = psmm.tile([128, 512], FP32, tag="mm", name="mm")
                nc.tensor.matmul(ps[:, :t1 - t0],
                                 vtmp_bd[:, ci * 128:(ci + 1) * 128],
                                 p1[:, t0:t1],
                                 start=True, stop=True)
                # rl broadcast [128, t]: rows = rlsp[head(partition)]
                psr = psmm.tile([128, 512], FP32, tag="mm", name="mmrl")
                nc.tensor.matmul(psr[:, :t1 - t0],
                                 headsel_t[:, ci * 128:(ci + 1) * 128],
                                 rlsp[:, t0:t1],
                                 start=True, stop=True)
                bct = stage.tile([128, 512], FP32, tag="bc", name="bct")
                nc.scalar.copy(bct[:, :t1 - t0], psr[:, :t1 - t0])
                nc.vector.tensor_mul(attnout[ci][:, t0:t1], ps[:, :t1 - t0],
                                     bct[:, :t1 - t0])

        # ---- S2T/P2T: temporal queries vs all local keys, token-major [T, 108]
        p2 = [small.tile([128, H * F], FP32R, tag=f"p2{tb}", name=f"p2{tb}") for tb in range(NTB)]
        for tb in range(NTB):
            p0, p1_ = tb * 128, min((tb + 1) * 128, T)
            ps = psmm.tile([128, 512], FP32, tag="mm", name="mm")
            for ci in range(NCH):
                nc.tensor.matmul(ps[:p1_ - p0, :H * F],
                                 k_t[ci][:, p0:p1_], qbd[ci][:],
                                 start=(ci == 0), stop=(ci == NCH - 1))
            nc.scalar.activation(p2[tb][:p1_ - p0, :], ps[:p1_ - p0, :H * F],
                                 mybir.ActivationFunctionType.Exp, scale=SCALE)
        # temporal keys masked out; CLS self-term kept only on even cores (via
        # host-zeroed e00mask), only for query j=0: rows 780..789 sit in block 6
        # at partitions 12..21
        # replacement block for the 9 temporal-key rows: all zero except the
        # CLS self-term at (row 0, cols h*9) kept on even cores via e00mask
        e00tmp = small.tile([1, H * F], FP32R, tag="e00tmp", name="e00tmp")
        znine = small.tile([F + 1, H * F], FP32R, tag="znine", name="znine")
        nc.sync.dma_start(e00tmp[:], p2[6][12:13, :])
        nc.sync.dma_start(znine[:], zeros[:F + 1, :H * F])
        with nc.allow_low_precision(reason="fp32r exp rounding intended"):
            nc.vector.tensor_mul(znine[0:1, :], e00tmp[:], e00_t[:])
        nc.sync.dma_start(p2[6][12:12 + F + 1, :], znine[:])

        # l2 partial [1,108] via ones-matmul over token blocks (fp32)
        l2 = small.tile([1, H * F], FP32, tag="l2", name="l2")
        ps_l2 = psst.tile([1, 512], FP32, tag="stat", name="stat")
        for tb in range(NTB):
            p0, p1_ = tb * 128, min((tb + 1) * 128, T)
            nc.tensor.matmul(ps_l2[:, :H * F], ones_t[:p1_ - p0, :],
                             p2[tb][:p1_ - p0, :],
                             start=(tb == 0), stop=(tb == NTB - 1))
        nc.scalar.copy(l2[:], ps_l2[:, :H * F])

        # O2 partial [9, 768]: per head accumulate over token blocks
        o2 = small.tile([F, C], FP32, tag="o2", name="o2")
        for h in range(H):
            ps = pso2.tile([F, 64], FP32, tag="o2", name="o2")
            for tb in range(NTB):
                p0, p1_ = tb * 128, min((tb + 1) * 128, T)
                nc.tensor.matmul(ps[:, :],
                                 p2[tb][:p1_ - p0, h * F:(h + 1) * F],
                                 v_t[tb][:p1_ - p0, h * 64:(h + 1) * 64],
                                 start=(tb == 0), stop=(tb == NTB - 1))
            nc.scalar.copy(o2[:, h * 64:(h + 1) * 64], ps[:])

        # ---- pairwise AllReduce of (l2, o2)
        cc_in1 = dram.tile([F, C], FP32, tag="cc_in1", name="cc_in1")
        cc_out1 = dram.tile([F, C], FP32, tag="cc_out1", name="cc_out1")
        cc_in2 = dram.tile([1, H * F], FP32, tag="cc_in2", name="cc_in2")
        cc_out2 = dram.tile([1, H * F], FP32, tag="cc_out2", name="cc_out2")
        groups = [[0, 1], [2, 3], [4, 5], [6, 7]]
        nc.sync.dma_start(cc_in1[:], o2[:])
        nc.sync.dma_start(cc_in2[:], l2[:])
        nc.gpsimd.collective_compute("AllReduce", mybir.AluOpType.add,
                                     replica_groups=groups,
                                     ins=[cc_in1.opt()], outs=[cc_out1.opt()])
        nc.gpsimd.collective_compute("AllReduce", mybir.AluOpType.add,
                                     replica_groups=groups,
                                     ins=[cc_in2.opt()], outs=[cc_out2.opt()])
        o2m = small.tile([F, C], FP32, tag="o2m", name="o2m")
        l2m = small.tile([1, H * F], FP32, tag="l2m", name="l2m")
        nc.sync.dma_start(o2m[:], cc_out1[:])
        nc.sync.dma_start(l2m[:], cc_out2[:])

        # normalize: o2m[j, (h,d)] /= l2m[h*9+j]; build rl2 [9,12] token-major
        rl2 = small.tile([1, H * F], FP32, tag="rl2", name="rl2")
        nc.vector.reciprocal(rl2[:], l2m[:])
        rl2jh = small.tile([F, H], FP32, tag="rl2jh", name="rl2jh")
        # DMA remap [1,(h,j)] -> [j, h]: per j row, gather h with stride F
        for j in range(F):
            nc.sync.dma_start(rl2jh[j:j + 1, :], rl2[:, j::F])
        o2n = small.tile([F, C], FP32, tag="o2n", name="o2n")
        for h in range(H):
            nc.vector.tensor_scalar_mul(o2n[:, h * 64:(h + 1) * 64],
                                        o2m[:, h * 64:(h + 1) * 64],
                                        rl2jh[:, h:h + 1])

        # transpose [9, 768] -> attnout cols 780..789 (6 PE transposes)
        for ci in range(NCH):
            pst = psmm.tile([128, 512], FP32, tag="mm", name="mm")
            nc.tensor.transpose(pst[:128, :F], o2n[:, ci * 128:(ci + 1) * 128],
                                id_t[:F, :F])
            nc.scalar.copy(attnout[ci][:, SPH:SPH + F], pst[:128, :F])

        # =========================================================
        # STAGE C: proj (+bias) -> projout
        # =========================================================
        projout = [act.tile([128, T], FP32R, tag=f"h{ci}", name=f"h{ci}") for ci in range(NCH)]  # reuse h
        for cg in range(0, C, 512):
            gw = min(512, C - cg)
            wts = [wpool.tile([128, 512], BF16, tag="w", name="w") for _ in range(NCH)]
            for ci in range(NCH):
                nc.sync.dma_start(wts[ci][:, :gw],
                                  projWt[ci * 128:(ci + 1) * 128, cg:cg + gw])
            for co in range(gw // 128):
                cout = cg + co * 128
                for (t0, t1) in T_TILES:
                    ps = psmm.tile([128, 512], FP32, tag="mm", name="mm")
                    for ci in range(NCH):
                        nc.tensor.matmul(
                            ps[:, :t1 - t0],
                            wts[ci][:, co * 128:(co + 1) * 128],
                            attnout[ci][:, t0:t1],
                            start=(ci == 0), stop=(ci == NCH - 1))
                    nc.scalar.activation(projout[cout // 128][:, t0:t1],
                                         ps[:, :t1 - t0],
                                         mybir.ActivationFunctionType.Identity,
                                         bias=pb_t[cout // 128][:, 0:1])

        # =========================================================
        # STAGE D: LN2 + MLP + residual
        # =========================================================
        h2 = [act.tile([128, T], BF16, tag=f"q{ci}", name=f"q{ci}") for ci in range(NCH)]  # reuse q
        bc_a2 = small.tile([128, T], FP32, tag="bca", name="bca2")
        bc_b2 = small.tile([128, T], FP32, tag="bcb", name="bcb2")
        layer_norm_fm(projout, h2, "k", bc_a2, bc_b2)  # scratch shares k slots

        hid = [big.tile([128, T], BF16, tag=f"hid{ci}", name=f"hid{ci}") for ci in range(NCH_FF)]
        # hid tiles [128, T] x24 = 75.8KB/partition; shares 'big' v slots via tags?
        # (v is [128, C] x7 = 21.5KB; keep both tags distinct: v dead after O2 but
        #  tile pool tags differ in shape; rely on pool bufs=1 per tag.)

        # fc1 + gelu
        for cg in range(0, DFF, 512):
            wts = [wpool.tile([128, 512], BF16, tag="w", name="w") for _ in range(NCH)]
            for ci in range(NCH):
                nc.sync.dma_start(wts[ci][:],
                                  fc1Wt[ci * 128:(ci + 1) * 128, cg:cg + 512])
            for co in range(4):
                cout = cg + co * 128
                for (t0, t1) in T_TILES:
                    ps = psmm.tile([128, 512], FP32, tag="mm", name="mm")
                    for ci in range(NCH):
                        nc.tensor.matmul(
                            ps[:, :t1 - t0],
                            wts[ci][:, co * 128:(co + 1) * 128],
                            h2[ci][:, t0:t1],
                            start=(ci == 0), stop=(ci == NCH - 1))
                    nc.scalar.activation(hid[cout // 128][:, t0:t1],
                                         ps[:, :t1 - t0],
                                         mybir.ActivationFunctionType.Gelu,
                                         bias=f1b_t[cout // 128][:, 0:1])

        # fc2 + bias + residual -> DMA out
        for cb in range(NCH):           # output 128-blocks
            wts = [wpool2.tile([128, 128], BF16, tag="w2", name="w2") for _ in range(NCH_FF)]
            for ci in range(NCH_FF):
                nc.sync.dma_start(wts[ci][:],
                                  fc2Wt[ci * 128:(ci + 1) * 128,
                                        cb * 128:(cb + 1) * 128])
            for (t0, t1) in T_TILES:
                ps = psmm.tile([128, 512], FP32, tag="mm", name="mm")
                for ci in range(NCH_FF):
                    nc.tensor.matmul(ps[:, :t1 - t0], wts[ci][:],
                                     hid[ci][:, t0:t1],
                                     start=(ci == 0), stop=(ci == NCH_FF - 1))
                st = stage.tile([128, 512], FP32, tag="out", name="out")
                nc.scalar.activation(st[:, :t1 - t0], ps[:, :t1 - t0],
                                     mybir.ActivationFunctionType.Identity,
                                     bias=f2b_t[cb][:, 0:1])
                nc.vector.tensor_add(st[:, :t1 - t0], st[:, :t1 - t0],
                                     projout[cb][:, t0:t1])
                nc.sync.dma_start(outT[cb * 128:(cb + 1) * 128, t0:t1],
                                  st[:, :t1 - t0])

    nc.compile()
    return nc


# ---------------- host side ----------------
_compiled = {}


def kernel(**inputs):
    x = np.ascontiguousarray(np.asarray(inputs['x'], np.float32))
    qkv_w = np.asarray(inputs['qkv_w'], np.float32)
    proj_w = np.asarray(inputs['proj_w'], np.float32)
    proj_b = np.asarray(inputs['proj_b'], np.float32)
    fc1_w = np.asarray(inputs['fc1_w'], np.float32)
    fc1_b = np.asarray(inputs['fc1_b'], np.float32)
    fc2_w = np.asarray(inputs['fc2_w'], np.float32)
    fc2_b = np.asarray(inputs['fc2_b'], np.float32)
    g = np.asarray(inputs['ln2_g'], np.float32)
    bb = np.asarray(inputs['ln2_b'], np.float32)

    import ml_dtypes
    bf16 = ml_dtypes.bfloat16
    qkvWt = np.ascontiguousarray(qkv_w.T).astype(bf16)    # [768, 2304]
    projWt = np.ascontiguousarray(proj_w.T).astype(bf16)  # [768, 768]
    fc1Wt = np.ascontiguousarray(fc1_w.T).astype(bf16)    # [768, 3072]
    fc2Wt = np.ascontiguousarray(fc2_w.T).astype(bf16)    # [3072, 768]
    gb = np.ascontiguousarray(np.stack([g, bb], 1))          # [768, 2]
    ones = np.ones((128, 1), np.float32)
    onesrow_np = np.ones((1, 128), np.float32)
    headsel_np = np.zeros((H, C), np.float32)
    for h in range(H):
        headsel_np[h, h * 64:(h + 1) * 64] = 1.0
    bd9 = np.zeros((H * F, H), np.float32)
    for h in range(H):
        bd9[h * F:(h + 1) * F, h] = 1.0
    ident = np.eye(128, dtype=np.float32)
    e00_even = np.zeros((1, H * F), np.float32)
    e00_even[0, ::F] = 1.0
    e00_odd = np.zeros((1, H * F), np.float32)

    in_maps = []
    for core in range(8):
        b, half = core // 2, core % 2
        sp = x[b, F + half * SPH: F + (half + 1) * SPH]     # [780, C]
        tmp = x[b, 0:F]                                      # [9, C]
        pad = np.zeros((1, C), np.float32)
        xT = np.ascontiguousarray(np.concatenate([sp, tmp, pad], 0).T)  # [C, 790]
        in_maps.append(dict(
            xT=xT, qkvWt=qkvWt, projWt=projWt, fc1Wt=fc1Wt, fc2Wt=fc2Wt,
            gb=gb, projB=proj_b.reshape(C, 1), fc1B=fc1_b.reshape(DFF, 1),
            fc2B=fc2_b.reshape(C, 1), ones=ones, onesrow=onesrow_np,
            zeros=np.zeros((128, C), np.float32),
            headsel=headsel_np, bd9=bd9, ident=ident,
            e00mask=(e00_even if half == 0 else e00_odd)))

    if 'nc' not in _compiled:
        _compiled['nc'] = build_kernel()
    nc = _compiled['nc']
    res = run_bass_kernel_spmd(nc, in_maps, list(range(8)))
    _compiled['last_result'] = res

    out = np.zeros((B, N, C), np.float32)
    for core in range(8):
        b, half = core // 2, core % 2
        oT = res.results[core]['outT']                       # [C, 789]
        if half == 0:
            out[b, 0:F] = oT[:, SPH:SPH + F].T
            out[b, F:F + SPH] = oT[:, 0:SPH].T
        else:
            out[b, F + SPH:N] = oT[:, 0:SPH].T
    return out


if __name__ == '__main__':
    from reference import setup_inputs, reference
    inputs = {k: np.asarray(v) for k, v in setup_inputs().items()}
    out = kernel(**inputs)
    print("kernel ran, out shape", out.shape)



# revision 10
# speedup vs baseline: 1.5317x; 1.5317x over previous
"""Trainium2 Bass kernel for the sparse-attention ('interact' mask) transformer block.

Reference computation (B=4, N=1569, C=768, H=12, d=64, Dff=3072, F=9):
    h   = LN(x);  qkv = h @ qkv_w.T;  sparse attention (spatial rows attend
    only to the 9 temporal tokens, temporal rows attend to the 1560 spatial
    tokens, CLS also to itself);  out = attn @ proj_w.T + proj_b;
    return out + MLP(LN(out))

Sharding: 8 cores = 4 batches x 2 halves. Each core owns one batch's half of
the 1560 spatial tokens (780) plus a replicated copy of the 9 temporal
tokens; local token layout is [780 spatial | 9 temporal | 1 pad].  The only
communication is one pairwise AllReduce(add) of flash-style partial softmax
stats packed as a single [10,768] tile (O2 partial in rows 0..8, l2 partial
in row 9), issued right after the attention partials and consumed ~70us
later by a small "temporal fixup" pass -- the spatial 780-column pipeline
(proj, LN2, fc1, fc2) never waits on it.

v2 structure (vs the 481us baseline): the tensor engine is kept continuously
busy (HAM clock-gate stays at 2.4GHz), all weights are DMA'd as large slabs
prefetched ahead of their GEMM, activations are bf16 (full PE rate even on
narrow moving dims), LN row chains are overlapped with independent PE work,
and the collective is fully off the critical path.

LN gamma/beta are folded host-side: gamma into the weight matrices, beta
into effective output biases (qkv bias is applied on q/k evictions; the v
bias is added post-attention, which is exact because softmax weights sum
to 1).
"""

import numpy as np
import sys
from contextlib import ExitStack

sys.path.insert(0, '/opt/trn_rl_repo')

import concourse.bass as bass
import concourse.bacc as bacc
import concourse.tile as tile
from concourse import mybir
from concourse.bass_utils import run_bass_kernel_spmd

# ---------------- problem constants (hardcoded per contract) ----------------
B, N, C = 4, 1569, 768
H, D = 12, 64
F = 9                    # temporal tokens (CLS + 8 frames)
DFF = 4 * C              # 3072
NSP = N - F              # 1560 spatial tokens
SPH = NSP // 2           # 780 spatial tokens per core
T = SPH + F + 1          # 790 local cols: [780 spatial | 9 temporal | 1 pad]
NCH = C // 128           # 6 feature chunks
NFF = DFF // 128         # 24 hidden chunks
NTB = (T + 127) // 128   # 7 token blocks (last = 22 rows)
SCALE = D ** -0.5
EPS = 1e-5

FP32 = mybir.dt.float32
FP32R = mybir.dt.float32r
BF16 = mybir.dt.bfloat16

TF = [(0, 512), (512, T)]        # full-width tiles (LN1, qkv, S1)
TS = [(0, 512), (512, SPH)]      # spatial-only tiles (O1, proj, LN2, fc1-T1..)
TX = (SPH, T)                    # temporal+pad fixup tile (10 cols)

# consts blob (bf16) column layout
CB_ONES = 0
CB_HSEL = 1            # headsel rows 0:12, cols 1:769
CB_BD9 = 769           # bd9 rows 0:108, cols 769:781
CB_BD9T = 781          # bd9T rows 0:12, cols 781:889
CB_MASK = 889          # [1,108] CLS-self mask row (row 0), cols 889:997
CB_W = 1000            # (cols 997:1000 pad)

# bias blob (fp32) column layout
BB_QB, BB_KB, BB_VB, BB_PB, BB_F1B, BB_F2B = 0, 6, 12, 18, 24, 48
BB_EPS = 54
BB_S2B = 55
BB_W = 56


def _r(ap):
    """View an fp32 AP as fp32r for full-rate PE matmuls."""
    return ap.bitcast(FP32R)


def build_kernel():
    nc = bacc.Bacc("TRN2", target_bir_lowering=False, debug=False,
                   num_devices=8)

    # ---------------- DRAM I/O ----------------
    xT = nc.dram_tensor("xT", [C, T], FP32R, kind="ExternalInput")
    qkvWt = nc.dram_tensor("qkvWt", [C, 3 * C], BF16, kind="ExternalInput")
    projWt = nc.dram_tensor("projWt", [C, C], BF16, kind="ExternalInput")
    fc1Wt = nc.dram_tensor("fc1Wt", [C, DFF], BF16, kind="ExternalInput")
    fc2Wt = nc.dram_tensor("fc2Wt", [DFF, C], BF16, kind="ExternalInput")
    cblob = nc.dram_tensor("cblob", [128, CB_W], BF16, kind="ExternalInput")
    bblob = nc.dram_tensor("bblob", [128, BB_W], FP32, kind="ExternalInput")
    ident = nc.dram_tensor("ident", [16, 16], FP32, kind="ExternalInput")
    ones32 = nc.dram_tensor("ones32", [128, 1], FP32R, kind="ExternalInput")
    onesrow32 = nc.dram_tensor("onesrow32", [1, 128], FP32R,
                               kind="ExternalInput")
    outT = nc.dram_tensor("outT", [C, T], FP32, kind="ExternalOutput")

    with tile.TileContext(nc) as tc, ExitStack() as ctx:
        act = ctx.enter_context(tc.tile_pool(name="act", bufs=1))
        big = ctx.enter_context(tc.tile_pool(name="big", bufs=1))
        wq = ctx.enter_context(tc.tile_pool(name="wq", bufs=1))
        wp = ctx.enter_context(tc.tile_pool(name="wp", bufs=1))
        small = ctx.enter_context(tc.tile_pool(name="small", bufs=1))
        rows = ctx.enter_context(tc.tile_pool(name="rows", bufs=1))
        scr = ctx.enter_context(tc.tile_pool(name="scr", bufs=1))
        stage = ctx.enter_context(tc.tile_pool(name="stage", bufs=2))
        psmm = ctx.enter_context(tc.tile_pool(name="psmm", bufs=3, space="PSUM"))
        psst = ctx.enter_context(tc.tile_pool(name="psst", bufs=3, space="PSUM"))
        pso2 = ctx.enter_context(tc.tile_pool(name="pso2", bufs=2, space="PSUM"))
        dram = ctx.enter_context(tc.tile_pool(name="dram", bufs=1, space="DRAM"))

        # ---------------- constants / biases / x / qkv+proj weights ----------
        cb = small.tile([128, CB_W], BF16, tag="cb", name="cb")
        nc.sync.dma_start(cb[:], cblob[:])
        bb = small.tile([128, BB_W], FP32, tag="bb", name="bb")
        nc.sync.dma_start(bb[:], bblob[:])
        id_t = small.tile([16, 16], FP32, tag="id", name="id")
        nc.sync.dma_start(id_t[:], ident[:])

        ones = cb[:, CB_ONES:CB_ONES + 1]                 # [128,1] bf16
        ones_f = small.tile([128, 1], FP32R, tag="ones32", name="ones32")
        nc.sync.dma_start(ones_f[:], ones32[:])
        onesrow_f = small.tile([1, 128], FP32R, tag="or32", name="or32")
        nc.sync.dma_start(onesrow_f[:], onesrow32[:])

        x_t = [act.tile([128, T], FP32R, tag=f"x{ci}", name=f"x{ci}")
               for ci in range(NCH)]
        for ci in range(NCH):
            nc.sync.dma_start(x_t[ci][:], xT[ci * 128:(ci + 1) * 128, :])

        # qkv weights: full-resident 6 x [128, 3072] (cols 0:2304 used now,
        # the same slots are later re-filled with fc1 weights)
        wq_t = [wq.tile([128, DFF], BF16, tag=f"wq{ci}", name=f"wq{ci}")
                for ci in range(NCH)]
        for ci in range(NCH):
            nc.sync.dma_start(wq_t[ci][:, 0:3 * C],
                              qkvWt[ci * 128:(ci + 1) * 128, :])
        # proj weights into the first 6 slots of the wp pool (later re-filled
        # with fc2 slabs)
        wp_t = [wp.tile([128, C], BF16, tag=f"wp{i}", name=f"wp{i}")
                for i in range(NCH)]
        for ci in range(NCH):
            nc.sync.dma_start(wp_t[ci][:], projWt[ci * 128:(ci + 1) * 128, :])

        # early memsets (no deps; keeps them off the critical path)
        kbd = [small.tile([128, H * F], BF16, tag=f"kbd{ci}", name=f"kbd{ci}")
               for ci in range(NCH)]
        qbd = [small.tile([128, H * F], BF16, tag=f"qbd{ci}", name=f"qbd{ci}")
               for ci in range(NCH)]
        vtmp_bd = small.tile([H * F, C], BF16, tag="vtmpbd", name="vtmpbd")
        for ci in range(NCH):
            nc.vector.memset(kbd[ci][:], 0)
            nc.vector.memset(qbd[ci][:], 0)
        nc.vector.memset(vtmp_bd[:], 0)

        # =========================================================
        # layernorm helper: stats chain for one tile -> (mu, al) row segments
        #   ps_sum/ps_sq: PSUM [1, w];  mu_t/al_t: SBUF [1, T] rows
        # =========================================================
        def ln_chain(ps_sum, ps_sq, mu_t, al_t, t0, t1):
            w = t1 - t0
            musq = rows.tile([1, 512], FP32, tag="row", name="musq", bufs=2)
            ex2 = rows.tile([1, 512], FP32, tag="row", name="ex2", bufs=2)
            nc.scalar.activation(mu_t[:, t0:t1], ps_sum[:, :w],
                                 mybir.ActivationFunctionType.Identity,
                                 scale=1.0 / C)
            nc.scalar.activation(musq[:, :w], ps_sum[:, :w],
                                 mybir.ActivationFunctionType.Square,
                                 scale=1.0 / C)
            nc.scalar.activation(ex2[:, :w], ps_sq[:, :w],
                                 mybir.ActivationFunctionType.Identity,
                                 scale=1.0 / C)
            nc.vector.tensor_sub(ex2[:, :w], ex2[:, :w], musq[:, :w])
            nc.scalar.activation(musq[:, :w], ex2[:, :w],
                                 mybir.ActivationFunctionType.Sqrt,
                                 bias=bb[0:1, BB_EPS:BB_EPS + 1])
            with nc.allow_low_precision(reason="softmax/LN reciprocal"):
                nc.vector.reciprocal(al_t[:, t0:t1], musq[:, :w])

        def ln_bcast(mu_t, al_t, bc_mu, bc_al, t0, t1):
            w = t1 - t0
            for (srow, bct) in ((mu_t, bc_mu), (al_t, bc_al)):
                psb = psmm.tile([128, 512], FP32, tag="mm", name="lnbc")
                nc.tensor.matmul(psb[:, :w], onesrow_f[:], srow[:, t0:t1],
                                 start=True, stop=True)
                nc.scalar.copy(bct[:, t0:t1], psb[:, :w])

        # =========================================================
        # STAGE A: LN1 (full 790 cols) + qkv
        # =========================================================
        # x^2 (bf16) on sq tiles (tag-shared with k: k is written later)
        sq = [act.tile([128, T], BF16, tag=f"k{ci}", name=f"sq{ci}")
              for ci in range(NCH)]
        for ci in range(NCH):
            nc.scalar.activation(sq[ci][:], x_t[ci][:],
                                 mybir.ActivationFunctionType.Square)
        mu_t = rows.tile([1, T], FP32R, tag="mu", name="mu1")
        al_t = rows.tile([1, T], FP32R, tag="al", name="al1")
        bc_mu = small.tile([128, T], BF16, tag="bcmu", name="bcmu1")
        bc_al = small.tile([128, T], BF16, tag="bcal", name="bcal1")
        stats = []
        for (t0, t1) in TF:
            w = t1 - t0
            ps_sum = psst.tile([12, 512], FP32, tag="st", name="sum")
            for ci in range(NCH):
                nc.tensor.matmul(ps_sum[0:1, :w], ones_f[:],
                                 x_t[ci][:, t0:t1],
                                 start=(ci == 0), stop=(ci == NCH - 1))
            ps_sq = psst.tile([12, 512], FP32, tag="st", name="sumsq")
            for ci in range(NCH):
                nc.tensor.matmul(ps_sq[0:1, :w], ones, sq[ci][:, t0:t1],
                                 start=(ci == 0), stop=(ci == NCH - 1))
            stats.append((ps_sum, ps_sq))
        for (t0, t1), (ps_sum, ps_sq) in zip(TF, stats):
            ln_chain(ps_sum[0:1], ps_sq[0:1], mu_t, al_t, t0, t1)
            ln_bcast(mu_t, al_t, bc_mu, bc_al, t0, t1)
        h_t = [act.tile([128, T], BF16, tag=f"h{ci}", name=f"h{ci}")
               for ci in range(NCH)]
        for ci in range(NCH):
            for (t0, t1) in TF:
                w = t1 - t0
                s = scr.tile([128, 512], FP32, tag="scr", name="lnscr")
                nc.vector.tensor_sub(s[:, :w], x_t[ci][:, t0:t1],
                                     bc_mu[:, t0:t1])
                nc.vector.tensor_mul(h_t[ci][:, t0:t1], s[:, :w],
                                     bc_al[:, t0:t1])

        # ---- q, k feature-major [C, T] bf16 (+ folded LN-beta bias) ----
        q_t = [act.tile([128, T], BF16, tag=f"q{ci}", name=f"q{ci}")
               for ci in range(NCH)]
        k_t = [act.tile([128, T], BF16, tag=f"k{ci}", name=f"k{ci}")
               for ci in range(NCH)]
        for (qk, dst, bbc) in ((0, q_t, BB_QB), (1, k_t, BB_KB)):
            for co in range(NCH):
                pss = []
                for (t0, t1) in TF:
                    ps = psmm.tile([128, 512], FP32, tag="mm", name="mm")
                    for ci in range(NCH):
                        nc.tensor.matmul(
                            ps[:, :t1 - t0],
                            wq_t[ci][:, qk * C + co * 128: qk * C + (co + 1) * 128],
                            h_t[ci][:, t0:t1],
                            start=(ci == 0), stop=(ci == NCH - 1))
                    pss.append(ps)
                for (t0, t1), ps in zip(TF, pss):
                    nc.scalar.activation(dst[co][:, t0:t1], ps[:, :t1 - t0],
                                         mybir.ActivationFunctionType.Identity,
                                         bias=bb[:, bbc + co:bbc + co + 1])
                # block-diag temporal slices for S1/S2 (2 heads per chunk)
                bd = kbd if qk == 1 else qbd
                for hh in (2 * co, 2 * co + 1):
                    po = (hh % 2) * 64
                    nc.vector.tensor_copy(
                        bd[co][po:po + 64, hh * F:(hh + 1) * F],
                        dst[co][po:po + 64, SPH:SPH + F])

        # ---- v token-major [T, C] bf16 (no bias: LN-beta's v-shift is added
        # post-attention where it is exact) ----
        v_t = [big.tile([128, C], BF16, tag=f"v{tb}", name=f"v{tb}")
               for tb in range(NTB)]
        for tb in range(NTB):
            p0, p1_ = tb * 128, min((tb + 1) * 128, T)
            rr = p1_ - p0
            pss = []
            for (c0, c1) in ((0, 512), (512, C)):
                ps = psmm.tile([128, 512], FP32, tag="mm", name="mmv")
                for ci in range(NCH):
                    nc.tensor.matmul(ps[:rr, :c1 - c0],
                                     h_t[ci][:, p0:p1_],
                                     wq_t[ci][:, 2 * C + c0: 2 * C + c1],
                                     start=(ci == 0), stop=(ci == NCH - 1))
                pss.append(ps)
            for (c0, c1), ps in zip(((0, 512), (512, C)), pss):
                nc.scalar.copy(v_t[tb][:rr, c0:c1], ps[:rr, :c1 - c0])
        # temporal v rows -> block-diag [108, 768] (12 cross-partition DMAs)
        for hh in range(H):
            nc.sync.dma_start(
                vtmp_bd[hh * F:(hh + 1) * F, hh * 64:(hh + 1) * 64],
                v_t[6][12:12 + F, hh * 64:(hh + 1) * 64])

        # =========================================================
        # STAGE B: sparse attention
        # =========================================================
        # S1: all local queries vs 9 temporal keys -> exp -> p1 [108, T] bf16
        p1 = small.tile([H * F, T], BF16, tag="p1", name="p1")
        for (t0, t1) in TF:
            ps = psmm.tile([128, 512], FP32, tag="mm", name="mms1")
            for ci in range(NCH):
                nc.tensor.matmul(ps[:H * F, :t1 - t0], kbd[ci][:],
                                 q_t[ci][:, t0:t1],
                                 start=(ci == 0), stop=(ci == NCH - 1))
            nc.scalar.activation(p1[:, t0:t1], ps[:H * F, :t1 - t0],
                                 mybir.ActivationFunctionType.Exp, scale=SCALE)
        # lsp[h,t] = sum_j p1[(h,j),t] ; rlsp = 1/lsp
        lsp = small.tile([12, T], FP32, tag="lsp", name="lsp")
        for (t0, t1) in TF:
            ps = psst.tile([12, 512], FP32, tag="st", name="lspps")
            nc.tensor.matmul(ps[:, :t1 - t0],
                             cb[0:H * F, CB_BD9:CB_BD9 + 12],
                             p1[:, t0:t1], start=True, stop=True)
            nc.scalar.copy(lsp[:, t0:t1], ps[:, :t1 - t0])
        with nc.allow_low_precision(reason="softmax reciprocal"):
            nc.vector.reciprocal(lsp[:], lsp[:])

        # S2T: temporal queries vs all local keys, token-major p2 [T, 108]
        p2 = [small.tile([128, H * F], BF16, tag=f"p2{tb}", name=f"p2{tb}")
              for tb in range(NTB)]
        for tb in range(NTB):
            p0, p1_ = tb * 128, min((tb + 1) * 128, T)
            rr = p1_ - p0
            ps = psmm.tile([128, 512], FP32, tag="mm", name="mms2")
            for ci in range(NCH):
                nc.tensor.matmul(ps[:rr, :H * F],
                                 k_t[ci][:, p0:p1_], qbd[ci][:],
                                 start=(ci == 0), stop=(ci == NCH - 1))
            # the last block holds the 9 temporal keys + pad at partitions
            # 12..21: a -1e4 pre-exp bias on partitions 13..21 zeroes them
            nc.scalar.activation(p2[tb][:rr, :], ps[:rr, :H * F],
                                 mybir.ActivationFunctionType.Exp, scale=SCALE,
                                 bias=(bb[0:rr, BB_S2B:BB_S2B + 1] if tb == 6
                                       else 0.0))
        # CLS-key row (partition 12): keep only the CLS self-term, and only on
        # even cores (DVE cannot address partition 12, so bounce via DMA)
        e00tmp = small.tile([1, H * F], BF16, tag="e00t", name="e00t")
        nc.sync.dma_start(e00tmp[:], p2[6][12:13, :])
        nc.vector.tensor_mul(e00tmp[:], e00tmp[:],
                             cb[0:1, CB_MASK:CB_MASK + H * F])
        nc.sync.dma_start(p2[6][12:13, :], e00tmp[:])

        # broadcast rlsp over (h,j) rows -> rb [108, T] bf16 (for O1 weights)
        rlsp_bf = small.tile([12, T], BF16, tag="rlspbf", name="rlspbf")
        nc.scalar.copy(rlsp_bf[:], lsp[:])
        rb = small.tile([H * F, T], BF16, tag="rb", name="rb")
        for (t0, t1) in TF:
            ps = psmm.tile([128, 512], FP32, tag="mm", name="mmrb")
            nc.tensor.matmul(ps[:H * F, :t1 - t0],
                             cb[0:12, CB_BD9T:CB_BD9T + H * F],
                             rlsp_bf[:, t0:t1], start=True, stop=True)
            nc.scalar.copy(rb[:, t0:t1], ps[:H * F, :t1 - t0])

        # l2 partial [1,108]
        l2row = small.tile([1, H * F], FP32, tag="l2", name="l2")
        ps_l2 = psst.tile([12, 512], FP32, tag="st", name="l2ps")
        for tb in range(NTB):
            p0, p1_ = tb * 128, min((tb + 1) * 128, T)
            nc.tensor.matmul(ps_l2[0:1, :H * F], cb[0:p1_ - p0, CB_ONES:CB_ONES + 1],
                             p2[tb][:p1_ - p0, :],
                             start=(tb == 0), stop=(tb == NTB - 1))
        nc.scalar.copy(l2row[:], ps_l2[0:1, :H * F])

        # O2 partial [9, 768]
        o2 = small.tile([F, C], FP32, tag="o2", name="o2")
        for hh in range(H):
            ps = pso2.tile([F, 64], FP32, tag="o2", name="o2ps")
            for tb in range(NTB):
                p0, p1_ = tb * 128, min((tb + 1) * 128, T)
                nc.tensor.matmul(ps[:, :],
                                 p2[tb][:p1_ - p0, hh * F:(hh + 1) * F],
                                 v_t[tb][:p1_ - p0, hh * 64:(hh + 1) * 64],
                                 start=(tb == 0), stop=(tb == NTB - 1))
            nc.scalar.copy(o2[:, hh * 64:(hh + 1) * 64], ps[:])

        # ---- single pairwise AllReduce of packed (o2 | l2), fully async ----
        cc_in = dram.tile([F + 1, C], FP32, tag="cc_in", name="cc_in")
        cc_out = dram.tile([F + 1, C], FP32, tag="cc_out", name="cc_out")
        nc.sync.dma_start(cc_in[0:F, :], o2[:])
        nc.sync.dma_start(cc_in[F:F + 1, 0:H * F], l2row[:])
        nc.gpsimd.collective_compute(
            "AllReduce", mybir.AluOpType.add,
            replica_groups=[[0, 1], [2, 3], [4, 5], [6, 7]],
            ins=[cc_in.opt()], outs=[cc_out.opt()])

        # ---- O1: spatial attention out (pre-normalized weights) ----
        for (t0, t1) in TS:
            nc.vector.tensor_mul(p1[:, t0:t1], p1[:, t0:t1], rb[:, t0:t1])
        attnout = [act.tile([128, T], BF16, tag=f"x{ci}", name=f"attn{ci}")
                   for ci in range(NCH)]
        for ci in range(NCH):
            nc.vector.memset(attnout[ci][:, SPH + F:T], 0)  # pad col stays finite
            for (t0, t1) in TS:
                ps = psmm.tile([128, 512], FP32, tag="mm", name="mmo1")
                nc.tensor.matmul(ps[:, :t1 - t0],
                                 vtmp_bd[:, ci * 128:(ci + 1) * 128],
                                 p1[:, t0:t1], start=True, stop=True)
                nc.scalar.activation(attnout[ci][:, t0:t1], ps[:, :t1 - t0],
                                     mybir.ActivationFunctionType.Identity,
                                     bias=bb[:, BB_VB + ci:BB_VB + ci + 1])

        # =========================================================
        # STAGE C: proj (spatial cols) + background weight prefetch
        # =========================================================
        # prefetch fc1 into the qkv slots and fc2 slabs into the wp pool
        wf1_t = [wq.tile([128, DFF], BF16, tag=f"wq{ci}", name=f"wf1{ci}")
                 for ci in range(NCH)]
        for ci in range(NCH):
            nc.sync.dma_start(wf1_t[ci][:],
                              fc1Wt[ci * 128:(ci + 1) * 128, :])
        wf2_t = [wp.tile([128, C], BF16, tag=f"wp{i}", name=f"wf2{i}")
                 for i in range(NFF)]
        for i in range(NCH, NFF):     # slots 6..23 are free immediately
            nc.sync.dma_start(wf2_t[i][:], fc2Wt[i * 128:(i + 1) * 128, :])

        projout = [act.tile([128, T], BF16, tag=f"h{ci}", name=f"po{ci}")
                   for ci in range(NCH)]
        sq2 = [act.tile([128, T], BF16, tag=f"k{ci}", name=f"sq2{ci}")
               for ci in range(NCH)]
        for co in range(NCH):
            pss = []
            for (t0, t1) in TS:
                ps = psmm.tile([128, 512], FP32, tag="mm", name="mmpj")
                for ci in range(NCH):
                    nc.tensor.matmul(
                        ps[:, :t1 - t0],
                        wp_t[ci][:, co * 128:(co + 1) * 128],
                        attnout[ci][:, t0:t1],
                        start=(ci == 0), stop=(ci == NCH - 1))
                pss.append(ps)
            for (t0, t1), ps in zip(TS, pss):
                nc.scalar.activation(projout[co][:, t0:t1], ps[:, :t1 - t0],
                                     mybir.ActivationFunctionType.Identity,
                                     bias=bb[:, BB_PB + co:BB_PB + co + 1])
                nc.scalar.activation(sq2[co][:, t0:t1], projout[co][:, t0:t1],
                                     mybir.ActivationFunctionType.Square)
        # =========================================================
        # STAGE D: LN2 (spatial) + fc1-T1
        # =========================================================
        mu2 = rows.tile([1, T], FP32R, tag="mu", name="mu2")
        al2 = rows.tile([1, T], FP32R, tag="al", name="al2")
        bc_mu2 = small.tile([128, T], BF16, tag="bcmu", name="bcmu2")
        bc_al2 = small.tile([128, T], BF16, tag="bcal", name="bcal2")
        h2 = [act.tile([128, T], BF16, tag=f"q{ci}", name=f"h2{ci}")
              for ci in range(NCH)]
        hid = [big.tile([128, T], BF16, tag=f"hid{i}", name=f"hid{i}")
               for i in range(NFF)]
        def ln2_apply(t0, t1):
            w = t1 - t0
            for ci in range(NCH):
                s = scr.tile([128, 512], FP32, tag="scr", name="ln2scr")
                nc.vector.tensor_sub(s[:, :w], projout[ci][:, t0:t1],
                                     bc_mu2[:, t0:t1])
                nc.vector.tensor_mul(h2[ci][:, t0:t1], s[:, :w],
                                     bc_al2[:, t0:t1])

        def fc1_tile(t0, t1):
            w = t1 - t0
            for fo in range(NFF):
                ps = psmm.tile([128, 512], FP32, tag="mm", name="mmf1")
                for ci in range(NCH):
                    nc.tensor.matmul(
                        ps[:, :w],
                        wf1_t[ci][:, fo * 128:(fo + 1) * 128],
                        h2[ci][:, t0:t1],
                        start=(ci == 0), stop=(ci == NCH - 1))
                nc.scalar.activation(hid[fo][:, t0:t1], ps[:, :w],
                                     mybir.ActivationFunctionType.Gelu,
                                     bias=bb[:, BB_F1B + fo:BB_F1B + fo + 1])

        stats2 = []
        for (t0, t1) in TS:
            w = t1 - t0
            ps_sum = psst.tile([12, 512], FP32, tag="st", name="sum2")
            for ci in range(NCH):
                nc.tensor.matmul(ps_sum[0:1, :w], ones,
                                 projout[ci][:, t0:t1],
                                 start=(ci == 0), stop=(ci == NCH - 1))
            ps_sq = psst.tile([12, 512], FP32, tag="st", name="sumsq2")
            for ci in range(NCH):
                nc.tensor.matmul(ps_sq[0:1, :w], ones, sq2[ci][:, t0:t1],
                                 start=(ci == 0), stop=(ci == NCH - 1))
            stats2.append((ps_sum, ps_sq))
        (t0, t1) = TS[0]
        ln_chain(stats2[0][0][0:1], stats2[0][1][0:1], mu2, al2, t0, t1)
        ln_bcast(mu2, al2, bc_mu2, bc_al2, t0, t1)
        ln2_apply(t0, t1)
        # T2 chain early (ACT runs it during fc1-T1); its PE broadcast is
        # emitted after fc1-T1 so the PE queue never head-blocks on it
        (t0, t1) = TS[1]
        ln_chain(stats2[1][0][0:1], stats2[1][1][0:1], mu2, al2, t0, t1)
        fc1_tile(*TS[0])
        ln_bcast(mu2, al2, bc_mu2, bc_al2, t0, t1)
        ln2_apply(t0, t1)

        # =========================================================
        # STAGE E: temporal fixup (consumes the AllReduce)
        # =========================================================
        o2m, l2m = o2, l2row
        nc.sync.dma_start(o2m[:], cc_out[0:F, :])
        nc.sync.dma_start(l2m[:], cc_out[F:F + 1, 0:H * F])
        with nc.allow_low_precision(reason="softmax reciprocal"):
            nc.vector.reciprocal(l2m[:], l2m[:])
        rl2hj = small.tile([12, 10], FP32, tag="rl2hj", name="rl2hj")
        nc.vector.memset(rl2hj[:], 0)
        for hh in range(H):
            nc.sync.dma_start(rl2hj[hh:hh + 1, 0:F],
                              l2m[:, hh * F:(hh + 1) * F])
        rl2hj_bf = small.tile([12, 10], BF16, tag="rl2hjbf", name="rl2hjbf")
        nc.scalar.copy(rl2hj_bf[:], rl2hj[:])
        # attnout temporal cols: transpose o2m, scale by 1/l2, add v-bias
        for ci in range(NCH):
            pst = psmm.tile([128, 512], FP32, tag="mm", name="mmtr")
            nc.tensor.transpose(pst[:128, :F],
                                o2m[:, ci * 128:(ci + 1) * 128],
                                id_t[:F, :F])
            psr = psmm.tile([128, 512], FP32, tag="mm", name="mmrl2")
            nc.tensor.matmul(psr[:, :10],
                             cb[0:12, CB_HSEL + ci * 128:CB_HSEL + (ci + 1) * 128],
                             rl2hj_bf[:], start=True, stop=True)
            rbc = scr.tile([128, 512], FP32, tag="scr", name="rbc")
            nc.scalar.copy(rbc[:, :F], psr[:, :F])
            nc.vector.tensor_mul(attnout[ci][:, SPH:SPH + F], pst[:128, :F],
                                 rbc[:, :F])
            nc.vector.tensor_scalar_add(attnout[ci][:, SPH:SPH + F],
                                        attnout[ci][:, SPH:SPH + F],
                                        bb[:, BB_VB + ci:BB_VB + ci + 1])
        # proj on the 10 temporal+pad cols
        (t0, t1) = TX
        for co in range(NCH):
            ps = psmm.tile([128, 512], FP32, tag="mm", name="mmpjf")
            for ci in range(NCH):
                nc.tensor.matmul(ps[:, :t1 - t0],
                                 wp_t[ci][:, co * 128:(co + 1) * 128],
                                 attnout[ci][:, t0:t1],
                                 start=(ci == 0), stop=(ci == NCH - 1))
            nc.scalar.activation(projout[co][:, t0:t1], ps[:, :t1 - t0],
                                 mybir.ActivationFunctionType.Identity,
                                 bias=bb[:, BB_PB + co:BB_PB + co + 1])
            nc.scalar.activation(sq2[co][:, t0:t1], projout[co][:, t0:t1],
                                 mybir.ActivationFunctionType.Square)
        # proj weights are dead now: fill fc2 slabs 0..5 (WAR on fixup proj)
        for i in range(NCH):
            nc.sync.dma_start(wf2_t[i][:], fc2Wt[i * 128:(i + 1) * 128, :])
        # LN2 on the 10 temporal+pad cols
        ps_sum = psst.tile([12, 512], FP32, tag="st", name="sumf")
        for ci in range(NCH):
            nc.tensor.matmul(ps_sum[0:1, :t1 - t0], ones,
                             projout[ci][:, t0:t1],
                             start=(ci == 0), stop=(ci == NCH - 1))
        ps_sq = psst.tile([12, 512], FP32, tag="st", name="sumsqf")
        for ci in range(NCH):
            nc.tensor.matmul(ps_sq[0:1, :t1 - t0], ones, sq2[ci][:, t0:t1],
                             start=(ci == 0), stop=(ci == NCH - 1))
        ln_chain(ps_sum[0:1], ps_sq[0:1], mu2, al2, t0, t1)
        ln_bcast(mu2, al2, bc_mu2, bc_al2, t0, t1)
        for ci in range(NCH):
            s = scr.tile([128, 512], FP32, tag="scr", name="lnfscr")
            nc.vector.tensor_sub(s[:, :t1 - t0], projout[ci][:, t0:t1],
                                 bc_mu2[:, t0:t1])
            nc.vector.tensor_mul(h2[ci][:, t0:t1], s[:, :t1 - t0],
                                 bc_al2[:, t0:t1])

        # =========================================================
        # STAGE F: fc1-T2 (cols 512:790, includes fixed-up temporal cols)
        # =========================================================
        (t0, t1) = (512, T)
        for fo in range(NFF):
            ps = psmm.tile([128, 512], FP32, tag="mm", name="mmf1b")
            for ci in range(NCH):
                nc.tensor.matmul(ps[:, :t1 - t0],
                                 wf1_t[ci][:, fo * 128:(fo + 1) * 128],
                                 h2[ci][:, t0:t1],
                                 start=(ci == 0), stop=(ci == NCH - 1))
            nc.scalar.activation(hid[fo][:, t0:t1], ps[:, :t1 - t0],
                                 mybir.ActivationFunctionType.Gelu,
                                 bias=bb[:, BB_F1B + fo:BB_F1B + fo + 1])

        # =========================================================
        # STAGE G: fc2 (full width) + residual + store
        # =========================================================
        for co in range(NCH):
            pss = []
            for (t0, t1) in TF:
                ps = psmm.tile([128, 512], FP32, tag="mm", name="mmf2")
                for ci in range(NFF):
                    nc.tensor.matmul(ps[:, :t1 - t0],
                                     wf2_t[ci][:, co * 128:(co + 1) * 128],
                                     hid[ci][:, t0:t1],
                                     start=(ci == 0), stop=(ci == NFF - 1))
                pss.append(ps)
            for (t0, t1), ps in zip(TF, pss):
                st = stage.tile([128, 512], FP32, tag="out", name="out")
                nc.scalar.activation(st[:, :t1 - t0], ps[:, :t1 - t0],
                                     mybir.ActivationFunctionType.Identity,
                                     bias=bb[:, BB_F2B + co:BB_F2B + co + 1])
                nc.vector.tensor_add(st[:, :t1 - t0], st[:, :t1 - t0],
                                     projout[co][:, t0:t1])
                nc.sync.dma_start(outT[co * 128:(co + 1) * 128, t0:t1],
                                  st[:, :t1 - t0])

    nc.compile()
    return nc


# ---------------- host side ----------------
_compiled = {}


def kernel(**inputs):
    x = np.ascontiguousarray(np.asarray(inputs['x'], np.float32))
    qkv_w = np.asarray(inputs['qkv_w'], np.float32)
    proj_w = np.asarray(inputs['proj_w'], np.float32)
    proj_b = np.asarray(inputs['proj_b'], np.float32)
    fc1_w = np.asarray(inputs['fc1_w'], np.float32)
    fc1_b = np.asarray(inputs['fc1_b'], np.float32)
    fc2_w = np.asarray(inputs['fc2_w'], np.float32)
    fc2_b = np.asarray(inputs['fc2_b'], np.float32)
    g = np.asarray(inputs['ln2_g'], np.float32)
    bb_ = np.asarray(inputs['ln2_b'], np.float32)

    import ml_dtypes
    bf16 = ml_dtypes.bfloat16

    # fold LN gamma into the weights, LN beta into effective output biases
    qkv_wg = qkv_w * g[None, :]
    fc1_wg = fc1_w * g[None, :]
    qkv_beta = qkv_wg @ bb_                     # [2304]
    fc1_b_eff = fc1_b + fc1_wg @ bb_            # [3072]

    qkvWt = np.ascontiguousarray(qkv_wg.T).astype(bf16)    # [768, 2304]
    projWt = np.ascontiguousarray(proj_w.T).astype(bf16)   # [768, 768]
    fc1Wt = np.ascontiguousarray(fc1_wg.T).astype(bf16)    # [768, 3072]
    fc2Wt = np.ascontiguousarray(fc2_w.T).astype(bf16)     # [3072, 768]

    # bias blob [128, BB_W] fp32
    bblob = np.zeros((128, BB_W), np.float32)
    bblob[:, BB_QB:BB_QB + 6] = qkv_beta[0:C].reshape(6, 128).T
    bblob[:, BB_KB:BB_KB + 6] = qkv_beta[C:2 * C].reshape(6, 128).T
    bblob[:, BB_VB:BB_VB + 6] = qkv_beta[2 * C:3 * C].reshape(6, 128).T
    bblob[:, BB_PB:BB_PB + 6] = proj_b.reshape(6, 128).T
    bblob[:, BB_F1B:BB_F1B + 24] = fc1_b_eff.reshape(24, 128).T
    bblob[:, BB_F2B:BB_F2B + 6] = fc2_b.reshape(6, 128).T
    bblob[:, BB_EPS] = EPS
    bblob[13:22, BB_S2B] = -1e4

    # consts blob [128, CB_W] bf16 (per-core: mask row differs by parity)
    def make_cblob(even):
        cb = np.zeros((128, CB_W), np.float32)
        cb[:, CB_ONES] = 1.0
        for hh in range(H):
            cb[hh, CB_HSEL + hh * 64:CB_HSEL + (hh + 1) * 64] = 1.0
        for hh in range(H):
            cb[hh * F:(hh + 1) * F, CB_BD9 + hh] = 1.0
            cb[hh, CB_BD9T + hh * F:CB_BD9T + (hh + 1) * F] = 1.0
        if even:
            for hh in range(H):
                cb[0, CB_MASK + hh * F] = 1.0
        return cb.astype(bf16)

    cblob_even = make_cblob(True)
    cblob_odd = make_cblob(False)
    ident = np.zeros((16, 16), np.float32)
    np.fill_diagonal(ident, 1.0)
    ones32 = np.ones((128, 1), np.float32)
    onesrow32 = np.ones((1, 128), np.float32)

    in_maps = []
    for core in range(8):
        b, half = core // 2, core % 2
        sp = x[b, F + half * SPH: F + (half + 1) * SPH]      # [780, C]
        tmp = x[b, 0:F]                                       # [9, C]
        pad = np.zeros((1, C), np.float32)
        xT = np.ascontiguousarray(np.concatenate([sp, tmp, pad], 0).T)  # [C, 790]
        in_maps.append(dict(
            xT=xT, qkvWt=qkvWt, projWt=projWt, fc1Wt=fc1Wt, fc2Wt=fc2Wt,
            bblob=bblob, cblob=(cblob_even if half == 0 else cblob_odd),
            ident=ident, ones32=ones32, onesrow32=onesrow32))

    if 'nc' not in _compiled:
        _compiled['nc'] = build_kernel()
    nc = _compiled['nc']
    res = run_bass_kernel_spmd(nc, in_maps, list(range(8)))
    _compiled['last_result'] = res

    out = np.zeros((B, N, C), np.float32)
    for core in range(8):
        b, half = core // 2, core % 2
        oT = res.results[core]['outT']                        # [C, 790]
        if half == 0:
            out[b, 0:F] = oT[:, SPH:SPH + F].T
            out[b, F:F + SPH] = oT[:, 0:SPH].T
        else:
            out[b, F + SPH:N] = oT[:, 0:SPH].T
    return out


if __name__ == '__main__':
    from reference import setup_inputs, reference
    inputs = {k: np.asarray(v) for k, v in setup_inputs().items()}
    out = kernel(**inputs)
    print("kernel ran, out shape", out.shape)


# revision 11
# speedup vs baseline: 1.8316x; 1.1958x over previous
"""Trainium2 Bass kernel for the sparse-attention ('interact' mask) transformer block.

Reference computation (B=4, N=1569, C=768, H=12, d=64, Dff=3072, F=9):
    h   = LN(x);  qkv = h @ qkv_w.T;  sparse attention (spatial rows attend
    only to the 9 temporal tokens, temporal rows attend to the 1560 spatial
    tokens, CLS also to itself);  out = attn @ proj_w.T + proj_b;
    return out + MLP(LN(out))

Sharding: 8 cores = 4 batches x 2 halves. Each core owns one batch's half of
the 1560 spatial tokens (780) plus a replicated copy of the 9 temporal
tokens; local token layout is [780 spatial | 9 temporal | 1 pad].  The only
communication is one pairwise AllReduce(add) of flash-style partial softmax
stats packed as a single [10,768] tile (O2 partial in rows 0..8, l2 partial
in row 9), issued right after the attention partials and consumed ~70us
later by a small "temporal fixup" pass -- the spatial 780-column pipeline
(proj, LN2, fc1, fc2) never waits on it.

v2 structure (vs the 481us baseline): the tensor engine is kept continuously
busy (HAM clock-gate stays at 2.4GHz), all weights are DMA'd as large slabs
prefetched ahead of their GEMM, activations are bf16 (full PE rate even on
narrow moving dims), LN row chains are overlapped with independent PE work,
and the collective is fully off the critical path.

LN gamma/beta are folded host-side: gamma into the weight matrices, beta
into effective output biases (qkv bias is applied on q/k evictions; the v
bias is added post-attention, which is exact because softmax weights sum
to 1).
"""

import numpy as np
import sys
from contextlib import ExitStack

sys.path.insert(0, '/opt/trn_rl_repo')

import concourse.bass as bass
import concourse.bacc as bacc
import concourse.tile as tile
from concourse import mybir
from concourse.bass_utils import run_bass_kernel_spmd

# ---------------- problem constants (hardcoded per contract) ----------------
B, N, C = 4, 1569, 768
H, D = 12, 64
F = 9                    # temporal tokens (CLS + 8 frames)
DFF = 4 * C              # 3072
NSP = N - F              # 1560 spatial tokens
SPH = NSP // 2           # 780 spatial tokens per core
T = SPH + F + 1          # 790 local cols: [780 spatial | 9 temporal | 1 pad]
NCH = C // 128           # 6 feature chunks
NFF = DFF // 128         # 24 hidden chunks
NTB = (T + 127) // 128   # 7 token blocks (last = 22 rows)
SCALE = D ** -0.5
EPS = 1e-5

FP32 = mybir.dt.float32
FP32R = mybir.dt.float32r
BF16 = mybir.dt.bfloat16

TF = [(0, 512), (512, T)]        # full-width tiles (LN1, qkv, S1)
TS = [(0, 512), (512, SPH)]      # spatial-only tiles (O1, proj, LN2, fc1-T1..)
TX = (SPH, T)                    # temporal+pad fixup tile (10 cols)

# consts blob (bf16) column layout
CB_ONES = 0
CB_HSEL = 1            # headsel rows 0:12, cols 1:769
CB_BD9 = 769           # bd9 rows 0:108, cols 769:781
CB_BD9T = 781          # bd9T rows 0:12, cols 781:889
CB_MASK = 889          # [1,108] CLS-self mask row (row 0), cols 889:997
CB_ONESROW = 997       # row 0 all-ones, cols 997:1125
CB_W = 1128

# bias blob (fp32) column layout
BB_QB, BB_KB, BB_VB, BB_PB, BB_F1B, BB_F2B = 0, 6, 12, 18, 24, 48
BB_EPS = 54
BB_S2B = 55
BB_W = 56


def _r(ap):
    """View an fp32 AP as fp32r for full-rate PE matmuls."""
    return ap.bitcast(FP32R)


def build_kernel():
    nc = bacc.Bacc("TRN2", target_bir_lowering=False, debug=False,
                   num_devices=8)

    # ---------------- DRAM I/O ----------------
    xT = nc.dram_tensor("xT", [C, T], FP32R, kind="ExternalInput")
    qkvWt = nc.dram_tensor("qkvWt", [C, 3 * C], BF16, kind="ExternalInput")
    projWt = nc.dram_tensor("projWt", [C, C], BF16, kind="ExternalInput")
    fc1Wt = nc.dram_tensor("fc1Wt", [C, DFF], BF16, kind="ExternalInput")
    fc2Wt = nc.dram_tensor("fc2Wt", [DFF, C], BF16, kind="ExternalInput")
    cblob = nc.dram_tensor("cblob", [128, CB_W], BF16, kind="ExternalInput")
    bblob = nc.dram_tensor("bblob", [128, BB_W], FP32, kind="ExternalInput")
    ident = nc.dram_tensor("ident", [16, 16], FP32, kind="ExternalInput")
    ones32 = nc.dram_tensor("ones32", [128, 1], FP32R, kind="ExternalInput")

    outT = nc.dram_tensor("outT", [C, T], FP32, kind="ExternalOutput")

    with tile.TileContext(nc) as tc, ExitStack() as ctx:
        act = ctx.enter_context(tc.tile_pool(name="act", bufs=1))
        big = ctx.enter_context(tc.tile_pool(name="big", bufs=1))
        wq = ctx.enter_context(tc.tile_pool(name="wq", bufs=1))
        wp = ctx.enter_context(tc.tile_pool(name="wp", bufs=1))
        small = ctx.enter_context(tc.tile_pool(name="small", bufs=1))
        rows = ctx.enter_context(tc.tile_pool(name="rows", bufs=1))
        scr = ctx.enter_context(tc.tile_pool(name="scr", bufs=1))
        stage = ctx.enter_context(tc.tile_pool(name="stage", bufs=2))
        psmm = ctx.enter_context(tc.tile_pool(name="psmm", bufs=4, space="PSUM"))
        psst = ctx.enter_context(tc.tile_pool(name="psst", bufs=2, space="PSUM"))
        pso2 = ctx.enter_context(tc.tile_pool(name="pso2", bufs=2, space="PSUM"))
        dram = ctx.enter_context(tc.tile_pool(name="dram", bufs=1, space="DRAM"))

        # ---------------- constants / biases / x / qkv+proj weights ----------
        cb = small.tile([128, CB_W], BF16, tag="cb", name="cb")
        nc.sync.dma_start(cb[:], cblob[:])
        bb = small.tile([128, BB_W], FP32, tag="bb", name="bb")
        nc.sync.dma_start(bb[:], bblob[:])
        id_t = small.tile([16, 16], FP32, tag="id", name="id")
        nc.sync.dma_start(id_t[:], ident[:])

        ones = cb[:, CB_ONES:CB_ONES + 1]                 # [128,1] bf16
        ones_f = small.tile([128, 1], FP32R, tag="ones32", name="ones32")
        nc.sync.dma_start(ones_f[:], ones32[:])
        onesrow_b = cb[0:1, CB_ONESROW:CB_ONESROW + 128]  # [1,128] bf16

        x_t = [act.tile([128, T], FP32R, tag=f"x{ci}", name=f"x{ci}")
               for ci in range(NCH)]
        for ci in range(NCH):
            nc.sync.dma_start(x_t[ci][:], xT[ci * 128:(ci + 1) * 128, :])

        # qkv weights: full-resident 6 x [128, 3072] (cols 0:2304 used now,
        # the same slots are later re-filled with fc1 weights)
        wq_t = [wq.tile([128, DFF], BF16, tag=f"wq{ci}", name=f"wq{ci}")
                for ci in range(NCH)]
        for part in range(3):      # q cols land first so the q GEMM starts early
            for ci in range(NCH):
                nc.sync.dma_start(
                    wq_t[ci][:, part * C:(part + 1) * C],
                    qkvWt[ci * 128:(ci + 1) * 128, part * C:(part + 1) * C])
        # proj weights into the first 6 slots of the wp pool (later re-filled
        # with fc2 slabs)
        wp_t = [wp.tile([128, C], BF16, tag=f"wp{i}", name=f"wp{i}")
                for i in range(NCH)]
        for ci in range(NCH):
            nc.sync.dma_start(wp_t[ci][:], projWt[ci * 128:(ci + 1) * 128, :])

        # early memsets (no deps; keeps them off the critical path)
        kbd = [small.tile([128, H * F], BF16, tag=f"kbd{ci}", name=f"kbd{ci}")
               for ci in range(NCH)]
        qbd = [small.tile([128, H * F], BF16, tag=f"qbd{ci}", name=f"qbd{ci}")
               for ci in range(NCH)]
        vtmp_bd = small.tile([H * F, C], BF16, tag="vtmpbd", name="vtmpbd")
        for ci in range(NCH):
            nc.vector.memset(kbd[ci][:], 0)
            nc.vector.memset(qbd[ci][:], 0)
        nc.vector.memset(vtmp_bd[:], 0)

        # =========================================================
        # layernorm helper: stats chain for one tile -> (mu, al) row segments
        #   ps_sum/ps_sq: PSUM [1, w];  mu_t/al_t: SBUF [1, T] rows
        # =========================================================
        def ln_chain(ps_sum, ps_sq, mu_t, al_t, t0, t1):
            w = t1 - t0
            musq = rows.tile([1, 512], FP32, tag="row", name="musq", bufs=2)
            ex2 = rows.tile([1, 512], FP32, tag="row", name="ex2", bufs=2)
            nc.scalar.activation(mu_t[:, t0:t1], ps_sum[:, :w],
                                 mybir.ActivationFunctionType.Identity,
                                 scale=1.0 / C)
            nc.scalar.activation(musq[:, :w], ps_sum[:, :w],
                                 mybir.ActivationFunctionType.Square,
                                 scale=1.0 / C)
            nc.scalar.activation(ex2[:, :w], ps_sq[:, :w],
                                 mybir.ActivationFunctionType.Identity,
                                 scale=1.0 / C)
            nc.vector.tensor_sub(ex2[:, :w], ex2[:, :w], musq[:, :w])
            nc.scalar.activation(musq[:, :w], ex2[:, :w],
                                 mybir.ActivationFunctionType.Sqrt,
                                 bias=bb[0:1, BB_EPS:BB_EPS + 1])
            nc.vector.reciprocal_approx_fast(ex2[:, :w], musq[:, :w])
            nc.scalar.copy(al_t[:, t0:t1], ex2[:, :w])

        def ln_bcast(mu_t, al_t, bc_mu, bc_al, t0, t1):
            w = t1 - t0
            for (srow, bct) in ((mu_t, bc_mu), (al_t, bc_al)):
                psb = psmm.tile([128, 512], FP32, tag="mm", name="lnbc")
                nc.tensor.matmul(psb[:, :w], onesrow_b, srow[:, t0:t1],
                                 start=True, stop=True)
                nc.scalar.copy(bct[:, t0:t1], psb[:, :w])

        # =========================================================
        # STAGE A: LN1 (full 790 cols) + qkv
        # =========================================================
        # x^2 (bf16) on sq tiles (tag-shared with k: k is written later)
        sq = [act.tile([128, T], BF16, tag=f"k{ci}", name=f"sq{ci}")
              for ci in range(NCH)]
        for ci in range(NCH):
            nc.scalar.activation(sq[ci][:], x_t[ci][:],
                                 mybir.ActivationFunctionType.Square)
        mu_t = rows.tile([1, T], BF16, tag="mu", name="mu1")
        al_t = rows.tile([1, T], BF16, tag="al", name="al1")
        bc_mu = small.tile([128, T], BF16, tag="bcmu", name="bcmu1")
        bc_al = small.tile([128, T], BF16, tag="bcal", name="bcal1")
        stats = []
        for (t0, t1) in TF:
            w = t1 - t0
            ps_sum = psst.tile([12, 512], FP32, tag="st", name="sum")
            for ci in range(NCH):
                nc.tensor.matmul(ps_sum[0:1, :w], ones_f[:],
                                 x_t[ci][:, t0:t1],
                                 start=(ci == 0), stop=(ci == NCH - 1))
            ps_sq = psst.tile([12, 512], FP32, tag="st", name="sumsq")
            for ci in range(NCH):
                nc.tensor.matmul(ps_sq[0:1, :w], ones, sq[ci][:, t0:t1],
                                 start=(ci == 0), stop=(ci == NCH - 1))
            stats.append((ps_sum, ps_sq))
        for (t0, t1), (ps_sum, ps_sq) in zip(TF, stats):
            ln_chain(ps_sum[0:1], ps_sq[0:1], mu_t, al_t, t0, t1)
            ln_bcast(mu_t, al_t, bc_mu, bc_al, t0, t1)
        h_t = [act.tile([128, T], BF16, tag=f"h{ci}", name=f"h{ci}")
               for ci in range(NCH)]
        for ci in range(NCH):
            for (t0, t1) in TF:
                w = t1 - t0
                s = scr.tile([128, 512], FP32, tag="scr", name="lnscr")
                nc.vector.tensor_sub(s[:, :w], x_t[ci][:, t0:t1],
                                     bc_mu[:, t0:t1])
                nc.vector.tensor_mul(h_t[ci][:, t0:t1], s[:, :w],
                                     bc_al[:, t0:t1])

        # ---- q, k feature-major [C, T] bf16 (+ folded LN-beta bias) ----
        q_t = [act.tile([128, T], BF16, tag=f"q{ci}", name=f"q{ci}")
               for ci in range(NCH)]
        k_t = [act.tile([128, T], BF16, tag=f"k{ci}", name=f"k{ci}")
               for ci in range(NCH)]
        for (qk, dst, bbc) in ((0, q_t, BB_QB), (1, k_t, BB_KB)):
            for co in range(NCH):
                pss = [psmm.tile([128, 512], FP32, tag="mm", name="mm")
                       for _ in TF]
                for ci in range(NCH):
                    for (t0, t1), ps in zip(TF, pss):
                        nc.tensor.matmul(
                            ps[:, :t1 - t0],
                            wq_t[ci][:, qk * C + co * 128: qk * C + (co + 1) * 128],
                            h_t[ci][:, t0:t1],
                            start=(ci == 0), stop=(ci == NCH - 1))
                for (t0, t1), ps in zip(TF, pss):
                    nc.scalar.activation(dst[co][:, t0:t1], ps[:, :t1 - t0],
                                         mybir.ActivationFunctionType.Identity,
                                         bias=bb[:, bbc + co:bbc + co + 1])
                # block-diag temporal slices for S1/S2 (2 heads per chunk)
                bd = kbd if qk == 1 else qbd
                for hh in (2 * co, 2 * co + 1):
                    po = (hh % 2) * 64
                    nc.vector.tensor_copy(
                        bd[co][po:po + 64, hh * F:(hh + 1) * F],
                        dst[co][po:po + 64, SPH:SPH + F])

        # ---- v token-major [T, C] bf16 (no bias: LN-beta's v-shift is added
        # post-attention where it is exact) ----
        v_t = [big.tile([128, C], BF16, tag=f"v{tb}", name=f"v{tb}")
               for tb in range(NTB)]
        for tb in range(NTB):
            p0, p1_ = tb * 128, min((tb + 1) * 128, T)
            rr = p1_ - p0
            pss = [psmm.tile([128, 512], FP32, tag="mm", name="mmv")
                   for _ in range(2)]
            for ci in range(NCH):
                for (c0, c1), ps in zip(((0, 512), (512, C)), pss):
                    nc.tensor.matmul(ps[:rr, :c1 - c0],
                                     h_t[ci][:, p0:p1_],
                                     wq_t[ci][:, 2 * C + c0: 2 * C + c1],
                                     start=(ci == 0), stop=(ci == NCH - 1))
            for (c0, c1), ps in zip(((0, 512), (512, C)), pss):
                nc.scalar.copy(v_t[tb][:rr, c0:c1], ps[:rr, :c1 - c0])
        # temporal v rows -> block-diag [108, 768] (12 cross-partition DMAs)
        for hh in range(H):
            nc.sync.dma_start(
                vtmp_bd[hh * F:(hh + 1) * F, hh * 64:(hh + 1) * 64],
                v_t[6][12:12 + F, hh * 64:(hh + 1) * 64])

        # =========================================================
        # STAGE B: sparse attention
        # =========================================================
        # S1: all local queries vs 9 temporal keys -> exp -> p1 [108, T] bf16
        p1 = small.tile([H * F, T], BF16, tag="p1", name="p1")
        ps_s1 = [psmm.tile([128, 512], FP32, tag="mm", name="mms1")
                 for _ in TF]
        for ci in range(NCH):
            for (t0, t1), ps in zip(TF, ps_s1):
                nc.tensor.matmul(ps[:H * F, :t1 - t0], kbd[ci][:],
                                 q_t[ci][:, t0:t1],
                                 start=(ci == 0), stop=(ci == NCH - 1))
        for (t0, t1), ps in zip(TF, ps_s1):
            nc.scalar.activation(p1[:, t0:t1], ps[:H * F, :t1 - t0],
                                 mybir.ActivationFunctionType.Exp, scale=SCALE)
        # lsp[h,t] = sum_j p1[(h,j),t] ; rlsp = 1/lsp
        lsp = small.tile([12, T], FP32, tag="lsp", name="lsp")
        for (t0, t1) in TF:
            ps = psst.tile([12, 512], FP32, tag="st", name="lspps")
            nc.tensor.matmul(ps[:, :t1 - t0],
                             cb[0:H * F, CB_BD9:CB_BD9 + 12],
                             p1[:, t0:t1], start=True, stop=True)
            nc.scalar.copy(lsp[:, t0:t1], ps[:, :t1 - t0])
        nc.vector.reciprocal_approx_fast(lsp[:], lsp[:])

        # S2T: temporal queries vs all local keys, token-major p2 [T, 108]
        p2 = [small.tile([128, H * F], BF16, tag=f"p2{tb}", name=f"p2{tb}")
              for tb in range(NTB)]
        for tb in range(NTB):
            p0, p1_ = tb * 128, min((tb + 1) * 128, T)
            rr = p1_ - p0
            ps = psmm.tile([128, 512], FP32, tag="mm", name="mms2")
            for ci in range(NCH):
                nc.tensor.matmul(ps[:rr, :H * F],
                                 k_t[ci][:, p0:p1_], qbd[ci][:],
                                 start=(ci == 0), stop=(ci == NCH - 1))
            # the last block holds the 9 temporal keys + pad at partitions
            # 12..21: a -1e4 pre-exp bias on partitions 13..21 zeroes them
            nc.scalar.activation(p2[tb][:rr, :], ps[:rr, :H * F],
                                 mybir.ActivationFunctionType.Exp, scale=SCALE,
                                 bias=(bb[0:rr, BB_S2B:BB_S2B + 1] if tb == 6
                                       else 0.0))
        # CLS-key row (partition 12): keep only the CLS self-term, and only on
        # even cores (DVE cannot address partition 12, so bounce via DMA)
        e00tmp = small.tile([1, H * F], BF16, tag="e00t", name="e00t")
        nc.sync.dma_start(e00tmp[:], p2[6][12:13, :])
        nc.vector.tensor_mul(e00tmp[:], e00tmp[:],
                             cb[0:1, CB_MASK:CB_MASK + H * F])
        nc.sync.dma_start(p2[6][12:13, :], e00tmp[:])

        # broadcast rlsp over (h,j) rows -> rb [108, T] bf16 (for O1 weights)
        rlsp_bf = small.tile([12, T], BF16, tag="rlspbf", name="rlspbf")
        nc.scalar.copy(rlsp_bf[:], lsp[:])
        rb = small.tile([H * F, T], BF16, tag="rb", name="rb")
        for (t0, t1) in TF:
            ps = psmm.tile([128, 512], FP32, tag="mm", name="mmrb")
            nc.tensor.matmul(ps[:H * F, :t1 - t0],
                             cb[0:12, CB_BD9T:CB_BD9T + H * F],
                             rlsp_bf[:, t0:t1], start=True, stop=True)
            nc.scalar.copy(rb[:, t0:t1], ps[:H * F, :t1 - t0])

        # l2 partial [1,108]
        l2row = small.tile([1, H * F], FP32, tag="l2", name="l2")
        ps_l2 = psst.tile([12, 512], FP32, tag="st", name="l2ps")
        for tb in range(NTB):
            p0, p1_ = tb * 128, min((tb + 1) * 128, T)
            nc.tensor.matmul(ps_l2[0:1, :H * F], cb[0:p1_ - p0, CB_ONES:CB_ONES + 1],
                             p2[tb][:p1_ - p0, :],
                             start=(tb == 0), stop=(tb == NTB - 1))
        nc.scalar.copy(l2row[:], ps_l2[0:1, :H * F])

        # O2 partial [9, 768]
        o2 = small.tile([F, C], FP32, tag="o2", name="o2")
        for hh in range(H):
            ps = pso2.tile([F, 64], FP32, tag="o2", name="o2ps")
            for tb in range(NTB):
                p0, p1_ = tb * 128, min((tb + 1) * 128, T)
                nc.tensor.matmul(ps[:, :],
                                 p2[tb][:p1_ - p0, hh * F:(hh + 1) * F],
                                 v_t[tb][:p1_ - p0, hh * 64:(hh + 1) * 64],
                                 start=(tb == 0), stop=(tb == NTB - 1))
            nc.scalar.copy(o2[:, hh * 64:(hh + 1) * 64], ps[:])

        # ---- single pairwise AllReduce of packed (o2 | l2), fully async ----
        cc_in = dram.tile([F + 1, C], FP32, tag="cc_in", name="cc_in")
        cc_out = dram.tile([F + 1, C], FP32, tag="cc_out", name="cc_out")
        nc.sync.dma_start(cc_in[0:F, :], o2[:])
        nc.sync.dma_start(cc_in[F:F + 1, 0:H * F], l2row[:])
        nc.gpsimd.collective_compute(
            "AllReduce", mybir.AluOpType.add,
            replica_groups=[[0, 1], [2, 3], [4, 5], [6, 7]],
            ins=[cc_in.opt()], outs=[cc_out.opt()])

        # ---- O1: spatial attention out (pre-normalized weights) ----
        for (t0, t1) in TS:
            nc.vector.tensor_mul(p1[:, t0:t1], p1[:, t0:t1], rb[:, t0:t1])
        attnout = [act.tile([128, T], BF16, tag=f"x{ci}", name=f"attn{ci}")
                   for ci in range(NCH)]
        for ci in range(NCH):
            nc.vector.memset(attnout[ci][:, SPH + F:T], 0)  # pad col stays finite
            pss = [psmm.tile([128, 512], FP32, tag="mm", name="mmo1")
                   for _ in TS]
            for (t0, t1), ps in zip(TS, pss):
                nc.tensor.matmul(ps[:, :t1 - t0],
                                 vtmp_bd[:, ci * 128:(ci + 1) * 128],
                                 p1[:, t0:t1], start=True, stop=True)
            for (t0, t1), ps in zip(TS, pss):
                nc.scalar.activation(attnout[ci][:, t0:t1], ps[:, :t1 - t0],
                                     mybir.ActivationFunctionType.Identity,
                                     bias=bb[:, BB_VB + ci:BB_VB + ci + 1])

        # =========================================================
        # STAGE C: proj (spatial cols) + background weight prefetch
        # =========================================================
        # prefetch fc1 into the qkv slots and fc2 slabs into the wp pool
        wf1_t = [wq.tile([128, DFF], BF16, tag=f"wq{ci}", name=f"wf1{ci}")
                 for ci in range(NCH)]
        for ci in range(NCH):
            nc.sync.dma_start(wf1_t[ci][:],
                              fc1Wt[ci * 128:(ci + 1) * 128, :])
        wf2_t = [wp.tile([128, C], BF16, tag=f"wp{i}", name=f"wf2{i}")
                 for i in range(NFF)]
        for i in range(NCH, NFF):     # slots 6..23 are free immediately
            nc.sync.dma_start(wf2_t[i][:], fc2Wt[i * 128:(i + 1) * 128, :])

        projout = [act.tile([128, T], BF16, tag=f"h{ci}", name=f"po{ci}")
                   for ci in range(NCH)]
        sq2 = [act.tile([128, T], BF16, tag=f"k{ci}", name=f"sq2{ci}")
               for ci in range(NCH)]
        for co in range(NCH):
            pss = [psmm.tile([128, 512], FP32, tag="mm", name="mmpj")
                   for _ in TS]
            for ci in range(NCH):
                for (t0, t1), ps in zip(TS, pss):
                    nc.tensor.matmul(
                        ps[:, :t1 - t0],
                        wp_t[ci][:, co * 128:(co + 1) * 128],
                        attnout[ci][:, t0:t1],
                        start=(ci == 0), stop=(ci == NCH - 1))
            for (t0, t1), ps in zip(TS, pss):
                nc.scalar.activation(projout[co][:, t0:t1], ps[:, :t1 - t0],
                                     mybir.ActivationFunctionType.Identity,
                                     bias=bb[:, BB_PB + co:BB_PB + co + 1])
                nc.scalar.activation(sq2[co][:, t0:t1], projout[co][:, t0:t1],
                                     mybir.ActivationFunctionType.Square)
        # =========================================================
        # STAGE D: LN2 (spatial) + fc1-T1
        # =========================================================
        mu2 = rows.tile([1, T], BF16, tag="mu", name="mu2")
        al2 = rows.tile([1, T], BF16, tag="al", name="al2")
        bc_mu2 = small.tile([128, T], BF16, tag="bcmu", name="bcmu2")
        bc_al2 = small.tile([128, T], BF16, tag="bcal", name="bcal2")
        h2 = [act.tile([128, T], BF16, tag=f"q{ci}", name=f"h2{ci}")
              for ci in range(NCH)]
        hid = [big.tile([128, T], BF16, tag=f"hid{i}", name=f"hid{i}")
               for i in range(NFF)]
        def ln2_apply(t0, t1):
            w = t1 - t0
            for ci in range(NCH):
                s = scr.tile([128, 512], FP32, tag="scr", name="ln2scr")
                nc.vector.tensor_sub(s[:, :w], projout[ci][:, t0:t1],
                                     bc_mu2[:, t0:t1])
                nc.vector.tensor_mul(h2[ci][:, t0:t1], s[:, :w],
                                     bc_al2[:, t0:t1])

        def fc1_tile(t0, t1):
            w = t1 - t0
            for fo in range(NFF):
                ps = psmm.tile([128, 512], FP32, tag="mm", name="mmf1")
                for ci in range(NCH):
                    nc.tensor.matmul(
                        ps[:, :w],
                        wf1_t[ci][:, fo * 128:(fo + 1) * 128],
                        h2[ci][:, t0:t1],
                        start=(ci == 0), stop=(ci == NCH - 1))
                nc.scalar.activation(hid[fo][:, t0:t1], ps[:, :w],
                                     mybir.ActivationFunctionType.Gelu,
                                     bias=bb[:, BB_F1B + fo:BB_F1B + fo + 1])

        stats2 = []
        for (t0, t1) in TS:
            w = t1 - t0
            ps_sum = psst.tile([12, 512], FP32, tag="st", name="sum2")
            for ci in range(NCH):
                nc.tensor.matmul(ps_sum[0:1, :w], ones,
                                 projout[ci][:, t0:t1],
                                 start=(ci == 0), stop=(ci == NCH - 1))
            ps_sq = psst.tile([12, 512], FP32, tag="st", name="sumsq2")
            for ci in range(NCH):
                nc.tensor.matmul(ps_sq[0:1, :w], ones, sq2[ci][:, t0:t1],
                                 start=(ci == 0), stop=(ci == NCH - 1))
            stats2.append((ps_sum, ps_sq))
        (t0, t1) = TS[0]
        ln_chain(stats2[0][0][0:1], stats2[0][1][0:1], mu2, al2, t0, t1)
        ln_bcast(mu2, al2, bc_mu2, bc_al2, t0, t1)
        ln2_apply(t0, t1)
        # T2 chain early (ACT runs it during fc1-T1); its PE broadcast is
        # emitted after fc1-T1 so the PE queue never head-blocks on it
        (t0, t1) = TS[1]
        ln_chain(stats2[1][0][0:1], stats2[1][1][0:1], mu2, al2, t0, t1)
        fc1_tile(*TS[0])
        ln_bcast(mu2, al2, bc_mu2, bc_al2, t0, t1)
        ln2_apply(t0, t1)

        # =========================================================
        # STAGE E: temporal fixup (consumes the AllReduce)
        # =========================================================
        o2m, l2m = o2, l2row
        nc.sync.dma_start(o2m[:], cc_out[0:F, :])
        nc.sync.dma_start(l2m[:], cc_out[F:F + 1, 0:H * F])
        nc.vector.reciprocal_approx_fast(l2m[:], l2m[:])
        rl2hj = small.tile([12, 10], FP32, tag="rl2hj", name="rl2hj")
        nc.vector.memset(rl2hj[:], 0)
        for hh in range(H):
            nc.sync.dma_start(rl2hj[hh:hh + 1, 0:F],
                              l2m[:, hh * F:(hh + 1) * F])
        rl2hj_bf = small.tile([12, 10], BF16, tag="rl2hjbf", name="rl2hjbf")
        nc.scalar.copy(rl2hj_bf[:], rl2hj[:])
        # attnout temporal cols: transpose o2m, scale by 1/l2, add v-bias
        for ci in range(NCH):
            pst = psmm.tile([128, 512], FP32, tag="mm", name="mmtr")
            nc.tensor.transpose(pst[:128, :F],
                                o2m[:, ci * 128:(ci + 1) * 128],
                                id_t[:F, :F])
            psr = psmm.tile([128, 512], FP32, tag="mm", name="mmrl2")
            nc.tensor.matmul(psr[:, :10],
                             cb[0:12, CB_HSEL + ci * 128:CB_HSEL + (ci + 1) * 128],
                             rl2hj_bf[:], start=True, stop=True)
            rbc = scr.tile([128, 512], FP32, tag="scr", name="rbc")
            nc.scalar.copy(rbc[:, :F], psr[:, :F])
            nc.vector.tensor_mul(attnout[ci][:, SPH:SPH + F], pst[:128, :F],
                                 rbc[:, :F])
            nc.vector.tensor_scalar_add(attnout[ci][:, SPH:SPH + F],
                                        attnout[ci][:, SPH:SPH + F],
                                        bb[:, BB_VB + ci:BB_VB + ci + 1])
        # proj on the 10 temporal+pad cols
        (t0, t1) = TX
        for co in range(NCH):
            ps = psmm.tile([128, 512], FP32, tag="mm", name="mmpjf")
            for ci in range(NCH):
                nc.tensor.matmul(ps[:, :t1 - t0],
                                 wp_t[ci][:, co * 128:(co + 1) * 128],
                                 attnout[ci][:, t0:t1],
                                 start=(ci == 0), stop=(ci == NCH - 1))
            nc.scalar.activation(projout[co][:, t0:t1], ps[:, :t1 - t0],
                                 mybir.ActivationFunctionType.Identity,
                                 bias=bb[:, BB_PB + co:BB_PB + co + 1])
            nc.scalar.activation(sq2[co][:, t0:t1], projout[co][:, t0:t1],
                                 mybir.ActivationFunctionType.Square)
        # proj weights are dead now: fill fc2 slabs 0..5 (WAR on fixup proj)
        for i in range(NCH):
            nc.sync.dma_start(wf2_t[i][:], fc2Wt[i * 128:(i + 1) * 128, :])
        # LN2 on the 10 temporal+pad cols
        ps_sum = psst.tile([12, 512], FP32, tag="st", name="sumf")
        for ci in range(NCH):
            nc.tensor.matmul(ps_sum[0:1, :t1 - t0], ones,
                             projout[ci][:, t0:t1],
                             start=(ci == 0), stop=(ci == NCH - 1))
        ps_sq = psst.tile([12, 512], FP32, tag="st", name="sumsqf")
        for ci in range(NCH):
            nc.tensor.matmul(ps_sq[0:1, :t1 - t0], ones, sq2[ci][:, t0:t1],
                             start=(ci == 0), stop=(ci == NCH - 1))
        ln_chain(ps_sum[0:1], ps_sq[0:1], mu2, al2, t0, t1)
        ln_bcast(mu2, al2, bc_mu2, bc_al2, t0, t1)
        for ci in range(NCH):
            s = scr.tile([128, 512], FP32, tag="scr", name="lnfscr")
            nc.vector.tensor_sub(s[:, :t1 - t0], projout[ci][:, t0:t1],
                                 bc_mu2[:, t0:t1])
            nc.vector.tensor_mul(h2[ci][:, t0:t1], s[:, :t1 - t0],
                                 bc_al2[:, t0:t1])

        # =========================================================
        # STAGE F: fc1-T2 (cols 512:790, includes fixed-up temporal cols)
        # =========================================================
        (t0, t1) = (512, T)
        for fo in range(NFF):
            ps = psmm.tile([128, 512], FP32, tag="mm", name="mmf1b")
            for ci in range(NCH):
                nc.tensor.matmul(ps[:, :t1 - t0],
                                 wf1_t[ci][:, fo * 128:(fo + 1) * 128],
                                 h2[ci][:, t0:t1],
                                 start=(ci == 0), stop=(ci == NCH - 1))
            nc.scalar.activation(hid[fo][:, t0:t1], ps[:, :t1 - t0],
                                 mybir.ActivationFunctionType.Gelu,
                                 bias=bb[:, BB_F1B + fo:BB_F1B + fo + 1])

        # =========================================================
        # STAGE G: fc2 (full width) + residual + store
        # =========================================================
        for co in range(NCH):
            pss = [psmm.tile([128, 512], FP32, tag="mm", name="mmf2")
                   for _ in TF]
            for ci in range(NFF):
                for (t0, t1), ps in zip(TF, pss):
                    nc.tensor.matmul(ps[:, :t1 - t0],
                                     wf2_t[ci][:, co * 128:(co + 1) * 128],
                                     hid[ci][:, t0:t1],
                                     start=(ci == 0), stop=(ci == NFF - 1))
            for (t0, t1), ps in zip(TF, pss):
                st = stage.tile([128, 512], FP32, tag="out", name="out")
                nc.scalar.activation(st[:, :t1 - t0], ps[:, :t1 - t0],
                                     mybir.ActivationFunctionType.Identity,
                                     bias=bb[:, BB_F2B + co:BB_F2B + co + 1])
                nc.vector.tensor_add(st[:, :t1 - t0], st[:, :t1 - t0],
                                     projout[co][:, t0:t1])
                nc.sync.dma_start(outT[co * 128:(co + 1) * 128, t0:t1],
                                  st[:, :t1 - t0])

    nc.compile()
    return nc


# ---------------- host side ----------------
_compiled = {}


def kernel(**inputs):
    x = np.ascontiguousarray(np.asarray(inputs['x'], np.float32))
    qkv_w = np.asarray(inputs['qkv_w'], np.float32)
    proj_w = np.asarray(inputs['proj_w'], np.float32)
    proj_b = np.asarray(inputs['proj_b'], np.float32)
    fc1_w = np.asarray(inputs['fc1_w'], np.float32)
    fc1_b = np.asarray(inputs['fc1_b'], np.float32)
    fc2_w = np.asarray(inputs['fc2_w'], np.float32)
    fc2_b = np.asarray(inputs['fc2_b'], np.float32)
    g = np.asarray(inputs['ln2_g'], np.float32)
    bb_ = np.asarray(inputs['ln2_b'], np.float32)

    import ml_dtypes
    bf16 = ml_dtypes.bfloat16

    # fold LN gamma into the weights, LN beta into effective output biases
    qkv_wg = qkv_w * g[None, :]
    fc1_wg = fc1_w * g[None, :]
    qkv_beta = qkv_wg @ bb_                     # [2304]
    fc1_b_eff = fc1_b + fc1_wg @ bb_            # [3072]

    qkvWt = np.ascontiguousarray(qkv_wg.T).astype(bf16)    # [768, 2304]
    projWt = np.ascontiguousarray(proj_w.T).astype(bf16)   # [768, 768]
    fc1Wt = np.ascontiguousarray(fc1_wg.T).astype(bf16)    # [768, 3072]
    fc2Wt = np.ascontiguousarray(fc2_w.T).astype(bf16)     # [3072, 768]

    # bias blob [128, BB_W] fp32
    bblob = np.zeros((128, BB_W), np.float32)
    bblob[:, BB_QB:BB_QB + 6] = qkv_beta[0:C].reshape(6, 128).T
    bblob[:, BB_KB:BB_KB + 6] = qkv_beta[C:2 * C].reshape(6, 128).T
    bblob[:, BB_VB:BB_VB + 6] = qkv_beta[2 * C:3 * C].reshape(6, 128).T
    bblob[:, BB_PB:BB_PB + 6] = proj_b.reshape(6, 128).T
    bblob[:, BB_F1B:BB_F1B + 24] = fc1_b_eff.reshape(24, 128).T
    bblob[:, BB_F2B:BB_F2B + 6] = fc2_b.reshape(6, 128).T
    bblob[:, BB_EPS] = EPS
    bblob[13:22, BB_S2B] = -1e4

    # consts blob [128, CB_W] bf16 (per-core: mask row differs by parity)
    def make_cblob(even):
        cb = np.zeros((128, CB_W), np.float32)
        cb[:, CB_ONES] = 1.0
        for hh in range(H):
            cb[hh, CB_HSEL + hh * 64:CB_HSEL + (hh + 1) * 64] = 1.0
        for hh in range(H):
            cb[hh * F:(hh + 1) * F, CB_BD9 + hh] = 1.0
            cb[hh, CB_BD9T + hh * F:CB_BD9T + (hh + 1) * F] = 1.0
        if even:
            for hh in range(H):
                cb[0, CB_MASK + hh * F] = 1.0
        cb[0, CB_ONESROW:CB_ONESROW + 128] = 1.0
        return cb.astype(bf16)

    cblob_even = make_cblob(True)
    cblob_odd = make_cblob(False)
    ident = np.zeros((16, 16), np.float32)
    np.fill_diagonal(ident, 1.0)
    ones32 = np.ones((128, 1), np.float32)

    in_maps = []
    for core in range(8):
        b, half = core // 2, core % 2
        sp = x[b, F + half * SPH: F + (half + 1) * SPH]      # [780, C]
        tmp = x[b, 0:F]                                       # [9, C]
        pad = np.zeros((1, C), np.float32)
        xT = np.ascontiguousarray(np.concatenate([sp, tmp, pad], 0).T)  # [C, 790]
        in_maps.append(dict(
            xT=xT, qkvWt=qkvWt, projWt=projWt, fc1Wt=fc1Wt, fc2Wt=fc2Wt,
            bblob=bblob, cblob=(cblob_even if half == 0 else cblob_odd),
            ident=ident, ones32=ones32))

    if 'nc' not in _compiled:
        _compiled['nc'] = build_kernel()
    nc = _compiled['nc']
    res = run_bass_kernel_spmd(nc, in_maps, list(range(8)))
    _compiled['last_result'] = res

    out = np.zeros((B, N, C), np.float32)
    for core in range(8):
        b, half = core // 2, core % 2
        oT = res.results[core]['outT']                        # [C, 790]
        if half == 0:
            out[b, 0:F] = oT[:, SPH:SPH + F].T
            out[b, F:F + SPH] = oT[:, 0:SPH].T
        else:
            out[b, F + SPH:N] = oT[:, 0:SPH].T
    return out


if __name__ == '__main__':
    from reference import setup_inputs, reference
    inputs = {k: np.asarray(v) for k, v in setup_inputs().items()}
    out = kernel(**inputs)
    print("kernel ran, out shape", out.shape)


# revision 13
# speedup vs baseline: 1.8871x; 1.0303x over previous
"""Trainium2 Bass kernel for the sparse-attention ('interact' mask) transformer block.

Reference computation (B=4, N=1569, C=768, H=12, d=64, Dff=3072, F=9):
    h   = LN(x);  qkv = h @ qkv_w.T;  sparse attention (spatial rows attend
    only to the 9 temporal tokens, temporal rows attend to the 1560 spatial
    tokens, CLS also to itself);  out = attn @ proj_w.T + proj_b;
    return out + MLP(LN(out))

Sharding: 8 cores = 4 batches x 2 halves. Each core owns one batch's half of
the 1560 spatial tokens (780) plus a replicated copy of the 9 temporal
tokens; local token layout is [780 spatial | 9 temporal | 1 pad].  The only
communication is one pairwise AllReduce(add) of flash-style partial softmax
stats packed as a single [10,768] tile (O2 partial in rows 0..8, l2 partial
in row 9), issued right after the attention partials and consumed ~70us
later by a small "temporal fixup" pass -- the spatial 780-column pipeline
(proj, LN2, fc1, fc2) never waits on it.

v2 structure (vs the 481us baseline): the tensor engine is kept continuously
busy (HAM clock-gate stays at 2.4GHz), all weights are DMA'd as large slabs
prefetched ahead of their GEMM, activations are bf16 (full PE rate even on
narrow moving dims), LN row chains are overlapped with independent PE work,
and the collective is fully off the critical path.

LN gamma/beta are folded host-side: gamma into the weight matrices, beta
into effective output biases (qkv bias is applied on q/k evictions; the v
bias is added post-attention, which is exact because softmax weights sum
to 1).
"""

import numpy as np
import sys
from contextlib import ExitStack

sys.path.insert(0, '/opt/trn_rl_repo')

import concourse.bass as bass
import concourse.bacc as bacc
import concourse.tile as tile
from concourse import mybir
from concourse.bass_utils import run_bass_kernel_spmd

# ---------------- problem constants (hardcoded per contract) ----------------
B, N, C = 4, 1569, 768
H, D = 12, 64
F = 9                    # temporal tokens (CLS + 8 frames)
DFF = 4 * C              # 3072
NSP = N - F              # 1560 spatial tokens
SPH = NSP // 2           # 780 spatial tokens per core
T = SPH + F + 1          # 790 local cols: [780 spatial | 9 temporal | 1 pad]
NCH = C // 128           # 6 feature chunks
NFF = DFF // 128         # 24 hidden chunks
NTB = (T + 127) // 128   # 7 token blocks (last = 22 rows)
SCALE = D ** -0.5
EPS = 1e-5

FP32 = mybir.dt.float32
FP32R = mybir.dt.float32r
BF16 = mybir.dt.bfloat16

TF = [(0, 512), (512, T)]        # full-width tiles (LN1, qkv, S1)
TS = [(0, 512), (512, SPH)]      # spatial-only tiles (O1, proj, LN2, fc1-T1..)
TX = (SPH, T)                    # temporal+pad fixup tile (10 cols)

# consts blob (bf16) column layout
CB_ONES = 0
CB_HSEL = 1            # headsel rows 0:12, cols 1:769
CB_BD9 = 769           # bd9 rows 0:108, cols 769:781
CB_BD9T = 781          # bd9T rows 0:12, cols 781:889
CB_MASK = 889          # [1,108] CLS-self mask row (row 0), cols 889:997
CB_ONESROW = 997       # row 0 all-ones, cols 997:1125
CB_W = 1128

# bias blob (fp32) column layout
BB_QB, BB_KB, BB_VB, BB_PB, BB_F1B, BB_F2B = 0, 6, 12, 18, 24, 48
BB_EPS = 54
BB_S2B = 55
BB_W = 56


def _r(ap):
    """View an fp32 AP as fp32r for full-rate PE matmuls."""
    return ap.bitcast(FP32R)


def build_kernel():
    nc = bacc.Bacc("TRN2", target_bir_lowering=False, debug=False,
                   num_devices=8)

    # ---------------- DRAM I/O ----------------
    xT = nc.dram_tensor("xT", [C, T], FP32R, kind="ExternalInput")
    qkvWt = nc.dram_tensor("qkvWt", [C, 3 * C], BF16, kind="ExternalInput")
    projWt = nc.dram_tensor("projWt", [C, C], BF16, kind="ExternalInput")
    fc1Wt = nc.dram_tensor("fc1Wt", [C, DFF], BF16, kind="ExternalInput")
    fc2Wt = nc.dram_tensor("fc2Wt", [DFF, C], BF16, kind="ExternalInput")
    cblob = nc.dram_tensor("cblob", [128, CB_W], BF16, kind="ExternalInput")
    bblob = nc.dram_tensor("bblob", [128, BB_W], FP32, kind="ExternalInput")
    ident = nc.dram_tensor("ident", [16, 16], FP32, kind="ExternalInput")
    ones32 = nc.dram_tensor("ones32", [128, 1], FP32R, kind="ExternalInput")

    outT = nc.dram_tensor("outT", [C, T], FP32, kind="ExternalOutput")

    with tile.TileContext(nc) as tc, ExitStack() as ctx:
        act = ctx.enter_context(tc.tile_pool(name="act", bufs=1))
        big = ctx.enter_context(tc.tile_pool(name="big", bufs=1))
        wq = ctx.enter_context(tc.tile_pool(name="wq", bufs=1))
        wp = ctx.enter_context(tc.tile_pool(name="wp", bufs=1))
        small = ctx.enter_context(tc.tile_pool(name="small", bufs=1))
        rows = ctx.enter_context(tc.tile_pool(name="rows", bufs=1))
        scr = ctx.enter_context(tc.tile_pool(name="scr", bufs=1))
        stage = ctx.enter_context(tc.tile_pool(name="stage", bufs=2))
        psmm = ctx.enter_context(tc.tile_pool(name="psmm", bufs=4, space="PSUM"))
        psst = ctx.enter_context(tc.tile_pool(name="psst", bufs=2, space="PSUM"))
        pso2 = ctx.enter_context(tc.tile_pool(name="pso2", bufs=2, space="PSUM"))
        dram = ctx.enter_context(tc.tile_pool(name="dram", bufs=1, space="DRAM"))

        # ---------------- constants / biases / x / qkv+proj weights ----------
        cb = small.tile([128, CB_W], BF16, tag="cb", name="cb")
        nc.sync.dma_start(cb[:], cblob[:])
        bb = small.tile([128, BB_W], FP32, tag="bb", name="bb")
        nc.sync.dma_start(bb[:], bblob[:])
        id_t = small.tile([16, 16], FP32, tag="id", name="id")
        nc.sync.dma_start(id_t[:], ident[:])

        ones = cb[:, CB_ONES:CB_ONES + 1]                 # [128,1] bf16
        ones_f = small.tile([128, 1], FP32R, tag="ones32", name="ones32")
        nc.sync.dma_start(ones_f[:], ones32[:])
        onesrow_b = cb[0:1, CB_ONESROW:CB_ONESROW + 128]  # [1,128] bf16

        x_t = [act.tile([128, T], FP32R, tag=f"x{ci}", name=f"x{ci}")
               for ci in range(NCH)]
        for ci in range(NCH):
            nc.sync.dma_start(x_t[ci][:], xT[ci * 128:(ci + 1) * 128, :])

        # qkv weights: full-resident 6 x [128, 3072] (cols 0:2304 used now,
        # the same slots are later re-filled with fc1 weights)
        wq_t = [wq.tile([128, DFF], BF16, tag=f"wq{ci}", name=f"wq{ci}")
                for ci in range(NCH)]
        for part in range(3):      # q cols land first so the q GEMM starts early
            for ci in range(NCH):
                nc.sync.dma_start(
                    wq_t[ci][:, part * C:(part + 1) * C],
                    qkvWt[ci * 128:(ci + 1) * 128, part * C:(part + 1) * C])
        # proj weights into the first 6 slots of the wp pool (later re-filled
        # with fc2 slabs)
        wp_t = [wp.tile([128, C], BF16, tag=f"wp{i}", name=f"wp{i}")
                for i in range(NCH)]
        for ci in range(NCH):
            nc.sync.dma_start(wp_t[ci][:], projWt[ci * 128:(ci + 1) * 128, :])

        # early memsets (no deps; keeps them off the critical path)
        kbd = [small.tile([128, H * F], BF16, tag=f"kbd{ci}", name=f"kbd{ci}")
               for ci in range(NCH)]
        qbd = [small.tile([128, H * F], BF16, tag=f"qbd{ci}", name=f"qbd{ci}")
               for ci in range(NCH)]
        vtmp_bd = small.tile([H * F, C], BF16, tag="vtmpbd", name="vtmpbd")
        for ci in range(NCH):
            nc.vector.memset(kbd[ci][:], 0)
            nc.vector.memset(qbd[ci][:], 0)
        nc.vector.memset(vtmp_bd[:], 0)

        # =========================================================
        # layernorm helper: stats chain for one tile -> (mu, al) row segments
        #   ps_sum/ps_sq: PSUM [1, w];  mu_t/al_t: SBUF [1, T] rows
        # =========================================================
        def ln_chain(ps_sum, ps_sq, mu_t, al_t, t0, t1):
            w = t1 - t0
            musq = rows.tile([1, 512], FP32, tag="row", name="musq", bufs=2)
            ex2 = rows.tile([1, 512], FP32, tag="row", name="ex2", bufs=2)
            nc.scalar.activation(mu_t[:, t0:t1], ps_sum[:, :w],
                                 mybir.ActivationFunctionType.Identity,
                                 scale=1.0 / C)
            nc.scalar.activation(musq[:, :w], ps_sum[:, :w],
                                 mybir.ActivationFunctionType.Square,
                                 scale=1.0 / C)
            nc.scalar.activation(ex2[:, :w], ps_sq[:, :w],
                                 mybir.ActivationFunctionType.Identity,
                                 scale=1.0 / C)
            nc.vector.tensor_sub(ex2[:, :w], ex2[:, :w], musq[:, :w])
            nc.scalar.activation(musq[:, :w], ex2[:, :w],
                                 mybir.ActivationFunctionType.Sqrt,
                                 bias=bb[0:1, BB_EPS:BB_EPS + 1])
            nc.vector.reciprocal_approx_fast(ex2[:, :w], musq[:, :w])
            nc.scalar.copy(al_t[:, t0:t1], ex2[:, :w])

        def ln_bcast(mu_t, al_t, bc_mu, bc_al, t0, t1):
            w = t1 - t0
            for (srow, bct) in ((mu_t, bc_mu), (al_t, bc_al)):
                psb = psmm.tile([128, 512], FP32, tag="mm", name="lnbc")
                nc.tensor.matmul(psb[:, :w], onesrow_b, srow[:, t0:t1],
                                 start=True, stop=True)
                nc.scalar.copy(bct[:, t0:t1], psb[:, :w])

        # =========================================================
        # STAGE A: LN1 (full 790 cols) + qkv
        # =========================================================
        # x^2 (bf16) on sq tiles (tag-shared with k: k is written later)
        sq = [act.tile([128, T], BF16, tag=f"k{ci}", name=f"sq{ci}")
              for ci in range(NCH)]
        for ci in range(NCH):
            nc.scalar.activation(sq[ci][:], x_t[ci][:],
                                 mybir.ActivationFunctionType.Square)
        mu_t = rows.tile([1, T], BF16, tag="mu", name="mu1")
        al_t = rows.tile([1, T], BF16, tag="al", name="al1")
        bc_mu = small.tile([128, T], BF16, tag="bcmu", name="bcmu1")
        bc_al = small.tile([128, T], BF16, tag="bcal", name="bcal1")
        stats = []
        for (t0, t1) in TF:
            w = t1 - t0
            ps_sum = psst.tile([12, 512], FP32, tag="st", name="sum")
            for ci in range(NCH):
                nc.tensor.matmul(ps_sum[0:1, :w], ones_f[:],
                                 x_t[ci][:, t0:t1],
                                 start=(ci == 0), stop=(ci == NCH - 1))
            ps_sq = psst.tile([12, 512], FP32, tag="st", name="sumsq")
            for ci in range(NCH):
                nc.tensor.matmul(ps_sq[0:1, :w], ones, sq[ci][:, t0:t1],
                                 start=(ci == 0), stop=(ci == NCH - 1))
            stats.append((ps_sum, ps_sq))
        for (t0, t1), (ps_sum, ps_sq) in zip(TF, stats):
            ln_chain(ps_sum[0:1], ps_sq[0:1], mu_t, al_t, t0, t1)
            ln_bcast(mu_t, al_t, bc_mu, bc_al, t0, t1)
        h_t = [act.tile([128, T], BF16, tag=f"h{ci}", name=f"h{ci}")
               for ci in range(NCH)]
        for ci in range(NCH):
            for (t0, t1) in TF:
                w = t1 - t0
                s = scr.tile([128, 512], FP32, tag="scr", name="lnscr")
                nc.vector.tensor_sub(s[:, :w], x_t[ci][:, t0:t1],
                                     bc_mu[:, t0:t1])
                nc.vector.tensor_mul(h_t[ci][:, t0:t1], s[:, :w],
                                     bc_al[:, t0:t1])

        # ---- q, k feature-major [C, T] bf16 (+ folded LN-beta bias) ----
        q_t = [act.tile([128, T], BF16, tag=f"q{ci}", name=f"q{ci}")
               for ci in range(NCH)]
        k_t = [act.tile([128, T], BF16, tag=f"k{ci}", name=f"k{ci}")
               for ci in range(NCH)]
        for (qk, dst, bbc) in ((0, q_t, BB_QB), (1, k_t, BB_KB)):
            for co in range(NCH):
                pss = [psmm.tile([128, 512], FP32, tag="mm", name="mm")
                       for _ in TF]
                for ci in range(NCH):
                    for (t0, t1), ps in zip(TF, pss):
                        nc.tensor.matmul(
                            ps[:, :t1 - t0],
                            wq_t[ci][:, qk * C + co * 128: qk * C + (co + 1) * 128],
                            h_t[ci][:, t0:t1],
                            start=(ci == 0), stop=(ci == NCH - 1))
                for (t0, t1), ps in zip(TF, pss):
                    nc.scalar.activation(dst[co][:, t0:t1], ps[:, :t1 - t0],
                                         mybir.ActivationFunctionType.Identity,
                                         bias=bb[:, bbc + co:bbc + co + 1])
                # block-diag temporal slices for S1/S2 (2 heads per chunk)
                bd = kbd if qk == 1 else qbd
                for hh in (2 * co, 2 * co + 1):
                    po = (hh % 2) * 64
                    nc.vector.tensor_copy(
                        bd[co][po:po + 64, hh * F:(hh + 1) * F],
                        dst[co][po:po + 64, SPH:SPH + F])

        # ---- v token-major [T, C] bf16 (no bias: LN-beta's v-shift is added
        # post-attention where it is exact) ----
        v_t = [big.tile([128, C], BF16, tag=f"v{tb}", name=f"v{tb}")
               for tb in range(NTB)]
        for tb in range(NTB):
            p0, p1_ = tb * 128, min((tb + 1) * 128, T)
            rr = p1_ - p0
            pss = [psmm.tile([128, 512], FP32, tag="mm", name="mmv")
                   for _ in range(2)]
            for ci in range(NCH):
                for (c0, c1), ps in zip(((0, 512), (512, C)), pss):
                    nc.tensor.matmul(ps[:rr, :c1 - c0],
                                     h_t[ci][:, p0:p1_],
                                     wq_t[ci][:, 2 * C + c0: 2 * C + c1],
                                     start=(ci == 0), stop=(ci == NCH - 1))
            for (c0, c1), ps in zip(((0, 512), (512, C)), pss):
                nc.scalar.copy(v_t[tb][:rr, c0:c1], ps[:rr, :c1 - c0])


        # =========================================================
        # STAGE B: sparse attention
        # =========================================================
        # S1: all local queries vs 9 temporal keys -> exp -> p1 [108, T] bf16
        p1 = small.tile([H * F, T], BF16, tag="p1", name="p1")
        ps_s1 = [psmm.tile([128, 512], FP32, tag="mm", name="mms1")
                 for _ in TF]
        for ci in range(NCH):
            for (t0, t1), ps in zip(TF, ps_s1):
                nc.tensor.matmul(ps[:H * F, :t1 - t0], kbd[ci][:],
                                 q_t[ci][:, t0:t1],
                                 start=(ci == 0), stop=(ci == NCH - 1))
        for (t0, t1), ps in zip(TF, ps_s1):
            nc.scalar.activation(p1[:, t0:t1], ps[:H * F, :t1 - t0],
                                 mybir.ActivationFunctionType.Exp, scale=SCALE)
        # lsp[h,t] = sum_j p1[(h,j),t] ; rlsp = 1/lsp
        lsp = small.tile([12, T], FP32, tag="lsp", name="lsp")
        for (t0, t1) in TF:
            ps = psst.tile([12, 512], FP32, tag="st", name="lspps")
            nc.tensor.matmul(ps[:, :t1 - t0],
                             cb[0:H * F, CB_BD9:CB_BD9 + 12],
                             p1[:, t0:t1], start=True, stop=True)
            nc.scalar.copy(lsp[:, t0:t1], ps[:, :t1 - t0])
        nc.vector.reciprocal_approx_fast(lsp[:], lsp[:])

        # S2T: temporal queries vs all local keys, token-major p2 [T, 108]
        p2 = [small.tile([128, H * F], BF16, tag=f"p2{tb}", name=f"p2{tb}")
              for tb in range(NTB)]
        for tb in range(NTB):
            p0, p1_ = tb * 128, min((tb + 1) * 128, T)
            rr = p1_ - p0
            ps = psmm.tile([128, 512], FP32, tag="mm", name="mms2")
            for ci in range(NCH):
                nc.tensor.matmul(ps[:rr, :H * F],
                                 k_t[ci][:, p0:p1_], qbd[ci][:],
                                 start=(ci == 0), stop=(ci == NCH - 1))
            # the last block holds the 9 temporal keys + pad at partitions
            # 12..21: a -1e4 pre-exp bias on partitions 13..21 zeroes them
            nc.scalar.activation(p2[tb][:rr, :], ps[:rr, :H * F],
                                 mybir.ActivationFunctionType.Exp, scale=SCALE,
                                 bias=(bb[0:rr, BB_S2B:BB_S2B + 1] if tb == 6
                                       else 0.0))
        # CLS-key row (partition 12): keep only the CLS self-term, and only on
        # even cores (DVE cannot address partition 12, so bounce via DMA)
        e00tmp = small.tile([1, H * F], BF16, tag="e00t", name="e00t")
        nc.gpsimd.dma_start(e00tmp[:], p2[6][12:13, :])
        nc.vector.tensor_mul(e00tmp[:], e00tmp[:],
                             cb[0:1, CB_MASK:CB_MASK + H * F])
        nc.gpsimd.dma_start(p2[6][12:13, :], e00tmp[:])
        # temporal v rows -> block-diag [108, 768] (12 cross-partition DMAs;
        # on the gpsimd queue so they never delay the CLS bounce or weights)
        for hh in range(H):
            nc.gpsimd.dma_start(
                vtmp_bd[hh * F:(hh + 1) * F, hh * 64:(hh + 1) * 64],
                v_t[6][12:12 + F, hh * 64:(hh + 1) * 64])

        # broadcast rlsp over (h,j) rows -> rb [108, T] bf16 (for O1 weights)
        rlsp_bf = small.tile([12, T], BF16, tag="rlspbf", name="rlspbf")
        nc.scalar.copy(rlsp_bf[:], lsp[:])
        rb = small.tile([H * F, T], BF16, tag="rb", name="rb")
        for (t0, t1) in TF:
            ps = psmm.tile([128, 512], FP32, tag="mm", name="mmrb")
            nc.tensor.matmul(ps[:H * F, :t1 - t0],
                             cb[0:12, CB_BD9T:CB_BD9T + H * F],
                             rlsp_bf[:, t0:t1], start=True, stop=True)
            nc.scalar.copy(rb[:, t0:t1], ps[:H * F, :t1 - t0])

        # l2 partial [1,108]
        l2row = small.tile([1, H * F], FP32, tag="l2", name="l2")
        ps_l2 = psst.tile([12, 512], FP32, tag="st", name="l2ps")
        for tb in range(NTB):
            p0, p1_ = tb * 128, min((tb + 1) * 128, T)
            nc.tensor.matmul(ps_l2[0:1, :H * F], cb[0:p1_ - p0, CB_ONES:CB_ONES + 1],
                             p2[tb][:p1_ - p0, :],
                             start=(tb == 0), stop=(tb == NTB - 1))
        nc.scalar.copy(l2row[:], ps_l2[0:1, :H * F])

        # O2 partial [9, 768]
        o2 = small.tile([F, C], FP32, tag="o2", name="o2")
        for hh in range(H):
            ps = pso2.tile([F, 64], FP32, tag="o2", name="o2ps")
            for tb in range(NTB):
                p0, p1_ = tb * 128, min((tb + 1) * 128, T)
                nc.tensor.matmul(ps[:, :],
                                 p2[tb][:p1_ - p0, hh * F:(hh + 1) * F],
                                 v_t[tb][:p1_ - p0, hh * 64:(hh + 1) * 64],
                                 start=(tb == 0), stop=(tb == NTB - 1))
            nc.scalar.copy(o2[:, hh * 64:(hh + 1) * 64], ps[:])

        # ---- single pairwise AllReduce of packed (o2 | l2), fully async ----
        cc_in = dram.tile([F + 1, C], FP32, tag="cc_in", name="cc_in")
        cc_out = dram.tile([F + 1, C], FP32, tag="cc_out", name="cc_out")
        nc.sync.dma_start(cc_in[0:F, :], o2[:])
        nc.sync.dma_start(cc_in[F:F + 1, 0:H * F], l2row[:])
        nc.gpsimd.collective_compute(
            "AllReduce", mybir.AluOpType.add,
            replica_groups=[[0, 1], [2, 3], [4, 5], [6, 7]],
            ins=[cc_in.opt()], outs=[cc_out.opt()])
        o2m, l2m = o2, l2row
        nc.gpsimd.dma_start(o2m[:], cc_out[0:F, :])
        nc.gpsimd.dma_start(l2m[:], cc_out[F:F + 1, 0:H * F])

        # ---- O1: spatial attention out (pre-normalized weights) ----
        for (t0, t1) in TS:
            nc.vector.tensor_mul(p1[:, t0:t1], p1[:, t0:t1], rb[:, t0:t1])
        attnout = [act.tile([128, T], BF16, tag=f"x{ci}", name=f"attn{ci}")
                   for ci in range(NCH)]
        for ci in range(NCH):
            nc.vector.memset(attnout[ci][:, SPH + F:T], 0)  # pad col stays finite
            pss = [psmm.tile([128, 512], FP32, tag="mm", name="mmo1")
                   for _ in TS]
            for (t0, t1), ps in zip(TS, pss):
                nc.tensor.matmul(ps[:, :t1 - t0],
                                 vtmp_bd[:, ci * 128:(ci + 1) * 128],
                                 p1[:, t0:t1], start=True, stop=True)
            for (t0, t1), ps in zip(TS, pss):
                nc.scalar.activation(attnout[ci][:, t0:t1], ps[:, :t1 - t0],
                                     mybir.ActivationFunctionType.Identity,
                                     bias=bb[:, BB_VB + ci:BB_VB + ci + 1])

        # =========================================================
        # STAGE C: proj (spatial cols) + background weight prefetch
        # =========================================================
        # prefetch fc1 into the qkv slots and fc2 slabs into the wp pool
        wf1_t = [wq.tile([128, DFF], BF16, tag=f"wq{ci}", name=f"wf1{ci}")
                 for ci in range(NCH)]
        for ci in range(NCH):
            nc.sync.dma_start(wf1_t[ci][:],
                              fc1Wt[ci * 128:(ci + 1) * 128, :])
        wf2_t = [wp.tile([128, C], BF16, tag=f"wp{i}", name=f"wf2{i}")
                 for i in range(NFF)]
        for i in range(NCH, NFF):     # slots 6..23 are free immediately
            nc.sync.dma_start(wf2_t[i][:], fc2Wt[i * 128:(i + 1) * 128, :])

        projout = [act.tile([128, T], BF16, tag=f"h{ci}", name=f"po{ci}")
                   for ci in range(NCH)]
        sq2 = [act.tile([128, T], BF16, tag=f"k{ci}", name=f"sq2{ci}")
               for ci in range(NCH)]
        for co in range(NCH):
            pss = [psmm.tile([128, 512], FP32, tag="mm", name="mmpj")
                   for _ in TS]
            for ci in range(NCH):
                for (t0, t1), ps in zip(TS, pss):
                    nc.tensor.matmul(
                        ps[:, :t1 - t0],
                        wp_t[ci][:, co * 128:(co + 1) * 128],
                        attnout[ci][:, t0:t1],
                        start=(ci == 0), stop=(ci == NCH - 1))
            for (t0, t1), ps in zip(TS, pss):
                nc.scalar.activation(projout[co][:, t0:t1], ps[:, :t1 - t0],
                                     mybir.ActivationFunctionType.Identity,
                                     bias=bb[:, BB_PB + co:BB_PB + co + 1])
                nc.scalar.activation(sq2[co][:, t0:t1], projout[co][:, t0:t1],
                                     mybir.ActivationFunctionType.Square)
        # =========================================================
        # STAGE D: LN2 (spatial) + fc1-T1
        # =========================================================
        mu2 = rows.tile([1, T], BF16, tag="mu", name="mu2")
        al2 = rows.tile([1, T], BF16, tag="al", name="al2")
        bc_mu2 = small.tile([128, T], BF16, tag="bcmu", name="bcmu2")
        bc_al2 = small.tile([128, T], BF16, tag="bcal", name="bcal2")
        h2 = [act.tile([128, T], BF16, tag=f"q{ci}", name=f"h2{ci}")
              for ci in range(NCH)]
        hid = [big.tile([128, T], BF16, tag=f"hid{i}", name=f"hid{i}")
               for i in range(NFF)]
        def ln2_apply(t0, t1):
            w = t1 - t0
            for ci in range(NCH):
                s = scr.tile([128, 512], FP32, tag="scr", name="ln2scr")
                nc.vector.tensor_sub(s[:, :w], projout[ci][:, t0:t1],
                                     bc_mu2[:, t0:t1])
                nc.vector.tensor_mul(h2[ci][:, t0:t1], s[:, :w],
                                     bc_al2[:, t0:t1])

        def fc1_tile(t0, t1):
            w = t1 - t0
            for fo in range(NFF):
                ps = psmm.tile([128, 512], FP32, tag="mm", name="mmf1")
                for ci in range(NCH):
                    nc.tensor.matmul(
                        ps[:, :w],
                        wf1_t[ci][:, fo * 128:(fo + 1) * 128],
                        h2[ci][:, t0:t1],
                        start=(ci == 0), stop=(ci == NCH - 1))
                nc.scalar.activation(hid[fo][:, t0:t1], ps[:, :w],
                                     mybir.ActivationFunctionType.Gelu,
                                     bias=bb[:, BB_F1B + fo:BB_F1B + fo + 1])

        stats2 = []
        for (t0, t1) in TS:
            w = t1 - t0
            ps_sum = psst.tile([12, 512], FP32, tag="st", name="sum2")
            for ci in range(NCH):
                nc.tensor.matmul(ps_sum[0:1, :w], ones,
                                 projout[ci][:, t0:t1],
                                 start=(ci == 0), stop=(ci == NCH - 1))
            ps_sq = psst.tile([12, 512], FP32, tag="st", name="sumsq2")
            for ci in range(NCH):
                nc.tensor.matmul(ps_sq[0:1, :w], ones, sq2[ci][:, t0:t1],
                                 start=(ci == 0), stop=(ci == NCH - 1))
            stats2.append((ps_sum, ps_sq))
        (t0, t1) = TS[0]
        ln_chain(stats2[0][0][0:1], stats2[0][1][0:1], mu2, al2, t0, t1)
        ln_bcast(mu2, al2, bc_mu2, bc_al2, t0, t1)
        ln2_apply(t0, t1)
        # T2 chain early (ACT runs it during fc1-T1); its PE broadcast is
        # emitted after fc1-T1 so the PE queue never head-blocks on it
        (t0, t1) = TS[1]
        ln_chain(stats2[1][0][0:1], stats2[1][1][0:1], mu2, al2, t0, t1)
        fc1_tile(*TS[0])
        # fixup normalizer prep runs on DVE/gpsimd while fc1-T1 owns the PE
        nc.vector.reciprocal_approx_fast(l2m[:], l2m[:])
        rl2hj = small.tile([12, 10], FP32, tag="rl2hj", name="rl2hj")
        nc.vector.memset(rl2hj[:], 0)
        for hh in range(H):
            nc.gpsimd.dma_start(rl2hj[hh:hh + 1, 0:F],
                                l2m[:, hh * F:(hh + 1) * F])
        rl2hj_bf = small.tile([12, 10], BF16, tag="rl2hjbf", name="rl2hjbf")
        nc.scalar.copy(rl2hj_bf[:], rl2hj[:])
        ln_bcast(mu2, al2, bc_mu2, bc_al2, t0, t1)
        ln2_apply(t0, t1)

        # =========================================================
        # STAGE E: temporal fixup (consumes the AllReduce)
        # =========================================================
        # attnout temporal cols: transpose o2m, scale by 1/l2, add v-bias
        for ci in range(NCH):
            pst = psmm.tile([128, 512], FP32, tag="mm", name="mmtr")
            nc.tensor.transpose(pst[:128, :F],
                                o2m[:, ci * 128:(ci + 1) * 128],
                                id_t[:F, :F])
            psr = psmm.tile([128, 512], FP32, tag="mm", name="mmrl2")
            nc.tensor.matmul(psr[:, :10],
                             cb[0:12, CB_HSEL + ci * 128:CB_HSEL + (ci + 1) * 128],
                             rl2hj_bf[:], start=True, stop=True)
            rbc = scr.tile([128, 512], FP32, tag="scr", name="rbc")
            nc.scalar.copy(rbc[:, :F], psr[:, :F])
            nc.vector.tensor_mul(attnout[ci][:, SPH:SPH + F], pst[:128, :F],
                                 rbc[:, :F])
            nc.vector.tensor_scalar_add(attnout[ci][:, SPH:SPH + F],
                                        attnout[ci][:, SPH:SPH + F],
                                        bb[:, BB_VB + ci:BB_VB + ci + 1])
        # proj on the 10 temporal+pad cols
        (t0, t1) = TX
        for co in range(NCH):
            ps = psmm.tile([128, 512], FP32, tag="mm", name="mmpjf")
            for ci in range(NCH):
                nc.tensor.matmul(ps[:, :t1 - t0],
                                 wp_t[ci][:, co * 128:(co + 1) * 128],
                                 attnout[ci][:, t0:t1],
                                 start=(ci == 0), stop=(ci == NCH - 1))
            nc.scalar.activation(projout[co][:, t0:t1], ps[:, :t1 - t0],
                                 mybir.ActivationFunctionType.Identity,
                                 bias=bb[:, BB_PB + co:BB_PB + co + 1])
            nc.vector.tensor_mul(sq2[co][:, t0:t1], projout[co][:, t0:t1],
                                  projout[co][:, t0:t1])
        # proj weights are dead now: fill fc2 slabs 0..5 (WAR on fixup proj)
        for i in range(NCH):
            nc.sync.dma_start(wf2_t[i][:], fc2Wt[i * 128:(i + 1) * 128, :])
        # LN2 on the 10 temporal+pad cols
        ps_sum = psst.tile([12, 512], FP32, tag="st", name="sumf")
        for ci in range(NCH):
            nc.tensor.matmul(ps_sum[0:1, :t1 - t0], ones,
                             projout[ci][:, t0:t1],
                             start=(ci == 0), stop=(ci == NCH - 1))
        ps_sq = psst.tile([12, 512], FP32, tag="st", name="sumsqf")
        for ci in range(NCH):
            nc.tensor.matmul(ps_sq[0:1, :t1 - t0], ones, sq2[ci][:, t0:t1],
                             start=(ci == 0), stop=(ci == NCH - 1))
        ln_chain(ps_sum[0:1], ps_sq[0:1], mu2, al2, t0, t1)
        ln_bcast(mu2, al2, bc_mu2, bc_al2, t0, t1)
        for ci in range(NCH):
            s = scr.tile([128, 512], FP32, tag="scr", name="lnfscr")
            nc.vector.tensor_sub(s[:, :t1 - t0], projout[ci][:, t0:t1],
                                 bc_mu2[:, t0:t1])
            nc.vector.tensor_mul(h2[ci][:, t0:t1], s[:, :t1 - t0],
                                 bc_al2[:, t0:t1])

        # =========================================================
        # STAGE F: fc1-T2 (cols 512:790, includes fixed-up temporal cols)
        # =========================================================
        (t0, t1) = (512, T)
        for fo in range(NFF):
            ps = psmm.tile([128, 512], FP32, tag="mm", name="mmf1b")
            for ci in range(NCH):
                nc.tensor.matmul(ps[:, :t1 - t0],
                                 wf1_t[ci][:, fo * 128:(fo + 1) * 128],
                                 h2[ci][:, t0:t1],
                                 start=(ci == 0), stop=(ci == NCH - 1))
            nc.scalar.activation(hid[fo][:, t0:t1], ps[:, :t1 - t0],
                                 mybir.ActivationFunctionType.Gelu,
                                 bias=bb[:, BB_F1B + fo:BB_F1B + fo + 1])

        # =========================================================
        # STAGE G: fc2 (full width) + residual + store
        # =========================================================
        for co in range(NCH):
            pss = [psmm.tile([128, 512], FP32, tag="mm", name="mmf2")
                   for _ in TF]
            for ci in range(NFF):
                for (t0, t1), ps in zip(TF, pss):
                    nc.tensor.matmul(ps[:, :t1 - t0],
                                     wf2_t[ci][:, co * 128:(co + 1) * 128],
                                     hid[ci][:, t0:t1],
                                     start=(ci == 0), stop=(ci == NFF - 1))
            for (t0, t1), ps in zip(TF, pss):
                st = stage.tile([128, 512], FP32, tag="out", name="out")
                nc.scalar.activation(st[:, :t1 - t0], ps[:, :t1 - t0],
                                     mybir.ActivationFunctionType.Identity,
                                     bias=bb[:, BB_F2B + co:BB_F2B + co + 1])
                nc.vector.tensor_add(st[:, :t1 - t0], st[:, :t1 - t0],
                                     projout[co][:, t0:t1])
                nc.sync.dma_start(outT[co * 128:(co + 1) * 128, t0:t1],
                                  st[:, :t1 - t0])

    nc.compile()
    return nc


# ---------------- host side ----------------
_compiled = {}


def kernel(**inputs):
    x = np.ascontiguousarray(np.asarray(inputs['x'], np.float32))
    qkv_w = np.asarray(inputs['qkv_w'], np.float32)
    proj_w = np.asarray(inputs['proj_w'], np.float32)
    proj_b = np.asarray(inputs['proj_b'], np.float32)
    fc1_w = np.asarray(inputs['fc1_w'], np.float32)
    fc1_b = np.asarray(inputs['fc1_b'], np.float32)
    fc2_w = np.asarray(inputs['fc2_w'], np.float32)
    fc2_b = np.asarray(inputs['fc2_b'], np.float32)
    g = np.asarray(inputs['ln2_g'], np.float32)
    bb_ = np.asarray(inputs['ln2_b'], np.float32)

    import ml_dtypes
    bf16 = ml_dtypes.bfloat16

    # fold LN gamma into the weights, LN beta into effective output biases
    qkv_wg = qkv_w * g[None, :]
    fc1_wg = fc1_w * g[None, :]
    qkv_beta = qkv_wg @ bb_                     # [2304]
    fc1_b_eff = fc1_b + fc1_wg @ bb_            # [3072]

    qkvWt = np.ascontiguousarray(qkv_wg.T).astype(bf16)    # [768, 2304]
    projWt = np.ascontiguousarray(proj_w.T).astype(bf16)   # [768, 768]
    fc1Wt = np.ascontiguousarray(fc1_wg.T).astype(bf16)    # [768, 3072]
    fc2Wt = np.ascontiguousarray(fc2_w.T).astype(bf16)     # [3072, 768]

    # bias blob [128, BB_W] fp32
    bblob = np.zeros((128, BB_W), np.float32)
    bblob[:, BB_QB:BB_QB + 6] = qkv_beta[0:C].reshape(6, 128).T
    bblob[:, BB_KB:BB_KB + 6] = qkv_beta[C:2 * C].reshape(6, 128).T
    bblob[:, BB_VB:BB_VB + 6] = qkv_beta[2 * C:3 * C].reshape(6, 128).T
    bblob[:, BB_PB:BB_PB + 6] = proj_b.reshape(6, 128).T
    bblob[:, BB_F1B:BB_F1B + 24] = fc1_b_eff.reshape(24, 128).T
    bblob[:, BB_F2B:BB_F2B + 6] = fc2_b.reshape(6, 128).T
    bblob[:, BB_EPS] = EPS
    bblob[13:22, BB_S2B] = -1e4

    # consts blob [128, CB_W] bf16 (per-core: mask row differs by parity)
    def make_cblob(even):
        cb = np.zeros((128, CB_W), np.float32)
        cb[:, CB_ONES] = 1.0
        for hh in range(H):
            cb[hh, CB_HSEL + hh * 64:CB_HSEL + (hh + 1) * 64] = 1.0
        for hh in range(H):
            cb[hh * F:(hh + 1) * F, CB_BD9 + hh] = 1.0
            cb[hh, CB_BD9T + hh * F:CB_BD9T + (hh + 1) * F] = 1.0
        if even:
            for hh in range(H):
                cb[0, CB_MASK + hh * F] = 1.0
        cb[0, CB_ONESROW:CB_ONESROW + 128] = 1.0
        return cb.astype(bf16)

    cblob_even = make_cblob(True)
    cblob_odd = make_cblob(False)
    ident = np.zeros((16, 16), np.float32)
    np.fill_diagonal(ident, 1.0)
    ones32 = np.ones((128, 1), np.float32)

    in_maps = []
    for core in range(8):
        b, half = core // 2, core % 2
        sp = x[b, F + half * SPH: F + (half + 1) * SPH]      # [780, C]
        tmp = x[b, 0:F]                                       # [9, C]
        pad = np.zeros((1, C), np.float32)
        xT = np.ascontiguousarray(np.concatenate([sp, tmp, pad], 0).T)  # [C, 790]
        in_maps.append(dict(
            xT=xT, qkvWt=qkvWt, projWt=projWt, fc1Wt=fc1Wt, fc2Wt=fc2Wt,
            bblob=bblob, cblob=(cblob_even if half == 0 else cblob_odd),
            ident=ident, ones32=ones32))

    if 'nc' not in _compiled:
        _compiled['nc'] = build_kernel()
    nc = _compiled['nc']
    res = run_bass_kernel_spmd(nc, in_maps, list(range(8)))
    _compiled['last_result'] = res

    out = np.zeros((B, N, C), np.float32)
    for core in range(8):
        b, half = core // 2, core % 2
        oT = res.results[core]['outT']                        # [C, 790]
        if half == 0:
            out[b, 0:F] = oT[:, SPH:SPH + F].T
            out[b, F:F + SPH] = oT[:, 0:SPH].T
        else:
            out[b, F + SPH:N] = oT[:, 0:SPH].T
    return out


if __name__ == '__main__':
    from reference import setup_inputs, reference
    inputs = {k: np.asarray(v) for k, v in setup_inputs().items()}
    out = kernel(**inputs)
    print("kernel ran, out shape", out.shape)
